# revision 21
# baseline (speedup 1.0000x reference)
# Trainium2 Bass kernel for nn_Cross_Transformer (dense_transformer).
#
# Sharding: 8 cores = 2 towers x 2 batches x 2 sequence-halves.
# Each core computes block0 (self-attention) in full (its inputs are permuted
# so its own half leads, keeping the program SPMD-uniform), then its half of
# block1 (cross-attention), pool, and final projection. No collectives.
#
# Layout: activations are feature-major [D on partitions, S on free].
# Q/K/V projections, attention scores and attn*V run in fp8e4m3 DoubleRow
# (2 contraction planes per instruction at 0.5 cycles/row). Q/K/V weights are
# pre-scaled by 32 to land in fp8's normal range; the 1/(32*32) shows up in
# the softmax exp scale and 1/32 is folded into the o-projection weights.
# Q/K features are permuted (prep-side) so each head's 64 dims sit as two
# 32-partition planes at free-stride 3*S, making score matmuls DoubleRow-able.
# o-proj, FFN, pool and final matmuls stay fp16 for accuracy.
# LayerNorm over D uses ones-column matmuls (values pre-scaled 1/D, 1/(D-1))
# and a short Square/Rsqrt chain; softmax denominators come from fp8 ones
# DoubleRow matmuls into psum rows 0/32, reciprocals stay on those rows, and
# per-head normalization is broadcast via K=1 matmuls into partition halves.

import math

import numpy as np

S = 1024
B = 2
D = 768
H = 12
DH = 64
EPS = 1e-6
SH = S // 2  # 512, per-core block1 rows
P = 128
ND = D // P  # 6 d-tiles
NS = S // P  # 8 s-tiles
NP = 3  # DoubleRow pair tiles per D-contraction
VW = D + DH  # V width: 12 head blocks + 64-wide ones block (denominators)
WS = 32.0  # fp8 weight pre-scale for q/k/v

_BUILT = {}


def _build_program():
    import concourse.bacc as bacc
    import concourse.tile as tile
    from concourse import mybir
    from concourse.masks import make_identity

    dt = mybir.dt
    f32 = dt.float32
    f16 = dt.float16
    f8 = dt.float8e4
    AF = mybir.ActivationFunctionType
    OP = mybir.AluOpType

    nc = bacc.Bacc("TRN2", target_bir_lowering=False, debug=False, num_devices=8)

    din = {}

    def dram_in(name, shape, dty):
        din[name] = nc.dram_tensor(name, list(shape), dty, kind="ExternalInput")
        return din[name]

    dram_in("srcT", (D, S), f16)
    dram_in("srcT8", (D, S), f8)
    dram_in("s1T", (D, SH), f16)
    dram_in("s1T8", (D, SH), f8)
    for li in (0, 1):
        dram_in(f"l{li}_qT8", (D, D), f8)
        dram_in(f"l{li}_kT8", (D, D), f8)
        dram_in(f"l{li}_vT8", (D + 2, VW), f8)
        dram_in(f"l{li}_oT", (D, D), f16)
        dram_in(f"l{li}_w1T", (D, D), f16)
        dram_in(f"l{li}_w2T", (D, D), f16)
    dram_in("spar", (P, 19, ND), f32)
    dram_in("pwT", (S + 1, S), f16)
    dram_in("finT", (2 * D, D), f16)

    outT = nc.dram_tensor("outT", [D, SH], f32, kind="ExternalOutput")
    scr1 = nc.dram_tensor("scr1", [SH * D], f16, kind="Internal")
    scr2 = nc.dram_tensor("scr2", [SH * D], f16, kind="Internal")

    with tile.TileContext(nc) as tc:
        _emit(nc, tc, tile, dt, AF, OP, din, outT, scr1, scr2, make_identity)

    nc.compile()
    return nc


def _emit(nc, tc, tile, dt, AF, OP, din, outT, scr1, scr2, make_identity):
    f32 = dt.float32
    f16 = dt.float16
    f8 = dt.float8e4
    import contextlib

    DR = None
    from concourse import mybir

    DR = mybir.MatmulPerfMode.DoubleRow
    EXS = 0.125 / (WS * WS)  # softmax exp input scale

    es = contextlib.ExitStack()
    with es:
        persist = es.enter_context(tc.tile_pool(name="persist", bufs=1))
        wp = es.enter_context(tc.tile_pool(name="wp", bufs=18))
        wp8 = es.enter_context(tc.tile_pool(name="wp8", bufs=7))
        psA = es.enter_context(tc.tile_pool(name="psA", bufs=2, space="PSUM"))
        psC = es.enter_context(tc.tile_pool(name="psC", bufs=2, space="PSUM"))
        psS = es.enter_context(tc.tile_pool(name="psS", bufs=2, space="PSUM"))
        expool = es.enter_context(tc.tile_pool(name="expool", bufs=5))
        sqp = es.enter_context(tc.tile_pool(name="sqp", bufs=4))
        rbp = es.enter_context(tc.tile_pool(name="rbp", bufs=4))
        rsp = es.enter_context(tc.tile_pool(name="rsp", bufs=4))

        # --- constants ---
        ident = persist.tile([P, P], f16, name="ident")
        make_identity(nc, ident)
        ones_mean = persist.tile([P, 1], f16, name="ones_mean")
        nc.vector.memset(ones_mean, 1.0 / D)
        ones_var = persist.tile([P, 1], f16, name="ones_var")
        nc.vector.memset(ones_var, 1.0 / (D - 1))
        ones_r128 = persist.tile([1, P], f16, name="ones_r128")
        nc.vector.memset(ones_r128, 1.0)

        # --- small params (biases, LN): one DMA, sliced views ---
        SPAR = persist.tile([P, 19, ND], f32, name="SPAR")
        nc.sync.dma_start(out=SPAR, in_=din["spar"].ap())
        par = {}
        pnames = ("bq", "bk", "bo", "b1", "b2", "ag", "ab", "fg", "fb")
        for li in (0, 1):
            for bi, bn in enumerate(pnames):
                par[f"l{li}_{bn}"] = SPAR[:, 9 * li + bi, :]
        finb = SPAR[:, 18, :]

        # persistent activations
        FEATS = persist.tile([P, ND, S], f16, name="FEATS")
        F8 = persist.tile([P, 8, S], f8, name="F8")
        nc.vector.memset(F8[0:1, 6, :], 1.0)
        nc.vector.memset(F8[0:1, 7, :], 0.0)
        S1T = persist.tile([P, ND, SH], f16, name="S1T")
        nc.sync.dma_start(
            out=S1T, in_=din["s1T"].ap().rearrange("(j p) s -> p j s", p=P)
        )
        S1T8 = persist.tile([P, ND, SH], f8, name="S1T8")
        nc.sync.dma_start(
            out=S1T8, in_=din["s1T8"].ap().rearrange("(j p) s -> p j s", p=P)
        )
        CTX1p = persist.tile([P, ND, SH], f16, name="CTX1p")
        Q1 = persist.tile([P, 2, NP, SH], f8, name="Q1")
        rb_t = persist.tile([P, S], f16, name="rb_t")
        mrb_t = persist.tile([P, S], f16, name="mrb_t")
        # LN small stats rows
        lnt = persist.tile([1, S], f32, name="lnt")
        lnv = persist.tile([1, S], f32, name="lnv")
        lnr16 = persist.tile([1, S], f16, name="lnr16")
        lnmr16 = persist.tile([1, S], f16, name="lnmr16")

        def load_w(dram_h, nk):
            """fp16 weight k-tiles [P, D]."""
            ap = dram_h.ap()
            tiles = []
            for t in range(nk):
                wt = wp.tile([P, D], f16, tag="w", name=f"w_{dram_h.name}_{t}")
                nc.sync.dma_start(out=wt, in_=ap[t * P : (t + 1) * P, :])
                tiles.append(wt)
            return tiles

        def load_w8(dram_h, width):
            """fp8 DoubleRow pair tiles [P, 2, width]."""
            ap = dram_h.ap()
            tiles = []
            for t in range(NP):
                wt = wp8.tile([P, 2, width], f8, tag="w8", name=f"w8_{dram_h.name}_{t}")
                nc.sync.dma_start(
                    out=wt,
                    in_=ap[2 * t * P : (2 * t + 2) * P, :].rearrange(
                        "(two p) w -> p two w", two=2
                    ),
                )
                tiles.append(wt)
            return tiles

        def fm_proj8(x8, w8s, Sx, evac):
            """fp8 DR projection: out[m] = sum_t w8s[t][:, :, mslice].T (x) x8-pairs."""
            nch = Sx // 512
            for m in range(ND):
                ps = psA.tile([P, Sx], f32, tag="psA", name=f"ps8_m{m}")
                for c in range(nch):
                    sl = slice(512 * c, 512 * (c + 1))
                    for t in range(NP):
                        nc.tensor.matmul(
                            ps[:, sl],
                            w8s[t][:, :, m * P : (m + 1) * P],
                            x8[:, 2 * t : 2 * t + 2, sl],
                            start=(t == 0),
                            stop=(t == NP - 1),
                            perf_mode=DR,
                        )
                evac(m, ps, 0)

        def v_proj8(x8, vw8, vb8, V8):
            """fp8 DR V projection (row-major out, bias via ones/zero planes)."""
            for st in range(NS):
                ps = psA.tile([P, VW], f32, tag="psA", name=f"vps{st}")
                ssl = slice(st * P, (st + 1) * P)
                for c0, c1 in ((0, 512), (512, VW)):
                    for t in range(NP):
                        nc.tensor.matmul(
                            ps[:, c0:c1],
                            x8[:, 2 * t : 2 * t + 2, ssl],
                            vw8[t][:, :, c0:c1],
                            start=(t == 0),
                            stop=False,
                            perf_mode=DR,
                        )
                    nc.tensor.matmul(
                        ps[:, c0:c1],
                        x8[0:1, 6:8, ssl],
                        vb8[:, :, c0:c1],
                        start=False,
                        stop=True,
                        perf_mode=DR,
                    )
                with nc.allow_low_precision(reason="fp8 V"):
                    nc.vector.tensor_copy(V8[:, st // 2, st % 2, :], ps)

        def attention8(q0, Qt, Kt, V8, CTXdst, interleave=()):
            """Per-head attention over q-columns [q0, q0+512), fp8 DoubleRow.
            Head h -> CTXdst partitions 64*(h%2), d-tile h//2. Softmax
            denominators come from the shared ones-block in V8."""
            SQ = 512
            qsl = slice(q0, q0 + SQ)
            for h in range(H):
                hp, hb = h // 2, 64 * (h % 2)
                pb = 32 * (h % 4)
                j0 = h // 4
                ctx_ps = psC.tile([DH, SQ], f32, tag="psC", name=f"ctx{h}")
                den = None
                for t in range(NS // 2):
                    ex = expool.tile([P, 2, SQ], f8, tag="ex", name=f"ex{h}_{t}")
                    sc = psA.tile([P, 2 * SQ], f32, tag="psA", name=f"sc{h}_{t}")
                    for half in (0, 1):
                        st = 2 * t + half
                        nc.tensor.matmul(
                            sc[:, half * SQ : (half + 1) * SQ],
                            Kt[pb : pb + 32, :, j0, st * P : (st + 1) * P],
                            Qt[pb : pb + 32, :, j0, qsl],
                            start=True,
                            stop=True,
                            perf_mode=DR,
                            tile_position=(pb, 0),
                        )
                    nc.scalar.activation(ex, sc, AF.Exp, scale=EXS)
                    if t == 0:
                        den = psS.tile([DH, SQ], f32, tag="psS", name=f"den{h}")
                    nc.tensor.matmul(
                        ctx_ps,
                        V8[:, t, :, DH * h : DH * (h + 1)],
                        ex,
                        start=(t == 0),
                        stop=(t == NS // 2 - 1),
                        perf_mode=DR,
                    )
                    nc.tensor.matmul(
                        den,
                        V8[:, t, :, D : D + DH],
                        ex,
                        start=(t == 0),
                        stop=(t == NS // 2 - 1),
                        perf_mode=DR,
                    )
                rs = rbp.tile([DH, SQ], f16, tag="rb", name=f"rs{h}")
                with nc.allow_low_precision(reason="softmax recip"):
                    nc.vector.reciprocal(rs, den)
                if h % 2 == 0:
                    with nc.allow_low_precision(reason="ctx norm"):
                        nc.vector.tensor_mul(
                            CTXdst[0:DH, hp, qsl], ctx_ps, rs
                        )
                else:
                    cte = rbp.tile([DH, SQ], f16, tag="rb", name=f"cte{h}")
                    with nc.allow_low_precision(reason="ctx evac"):
                        nc.vector.tensor_copy(cte, ctx_ps)
                        nc.gpsimd.tensor_mul(
                            CTXdst[hb : hb + DH, hp, qsl], cte, rs
                        )
                if h < len(interleave):
                    interleave[h]()

        def layernorm(Zt, q0, Sx, out_fn):
            """LN over partitions(d) of Zt[:, :, q0:q0+Sx] (Bessel std)."""
            nch = Sx // 512
            for c in range(nch):
                sl = slice(q0 + 512 * c, q0 + 512 * (c + 1))
                sum_ps = psS.tile([1, 512], f32, tag="psS", name=f"lnsum{c}")
                for k in range(ND):
                    nc.tensor.matmul(
                        sum_ps,
                        ones_mean[:, 0:1],
                        Zt[:, k, sl],
                        start=(k == 0),
                        stop=(k == ND - 1),
                    )
                sq_ps = psS.tile([1, 512], f32, tag="psS", name=f"lnsq{c}")
                for k in range(ND):
                    sq = sqp.tile([P, 512], f16, tag="sq", name=f"sq{k}{c}")
                    nc.gpsimd.tensor_mul(sq, Zt[:, k, sl], Zt[:, k, sl])
                    nc.tensor.matmul(
                        sq_ps,
                        ones_var[:, 0:1],
                        sq,
                        start=(k == 0),
                        stop=(k == ND - 1),
                    )
                # var = E[z^2]*D/(D-1) - mean^2*D/(D-1); r = 1/sqrt(var)
                nc.scalar.activation(
                    lnt[:, sl], sum_ps, AF.Square,
                    scale=math.sqrt(D / (D - 1.0)),
                )
                nc.vector.tensor_sub(lnv[:, sl], sq_ps, lnt[:, sl])
                nc.vector.reciprocal(lnv[:, sl], lnv[:, sl])
                nc.scalar.activation(lnr16[:, sl], lnv[:, sl], AF.Sqrt)
                with nc.allow_low_precision(reason="ln mr fp16"):
                    nc.vector.tensor_mul(lnmr16[:, sl], sum_ps, lnr16[:, sl])
            for c in range(nch):
                sl = slice(q0 + 512 * c, q0 + 512 * (c + 1))
                rb_ps = psS.tile([P, 512], f32, tag="psS", name=f"rbps{c}")
                nc.tensor.matmul(
                    rb_ps, ones_r128, lnr16[0:1, sl], start=True, stop=True
                )
                nc.vector.tensor_copy(rb_t[:, sl], rb_ps)
                mrb_ps = psS.tile([P, 512], f32, tag="psS", name=f"mrbps{c}")
                nc.tensor.matmul(
                    mrb_ps, ones_r128, lnmr16[0:1, sl], start=True, stop=True
                )
                nc.vector.tensor_copy(mrb_t[:, sl], mrb_ps)
                for k in range(ND):
                    t1 = sqp.tile([P, 512], f16, tag="sq", name=f"ap{k}{c}")
                    nc.vector.tensor_mul(t1, Zt[:, k, sl], rb_t[:, sl])
                    nc.vector.tensor_sub(t1, t1, mrb_t[:, sl])
                    out_fn(k, sl, t1)

        # ================= BLOCK 0 (full S, self-attention on src) =========
        with tc.tile_pool(name="b0a", bufs=5) as act6, tc.tile_pool(
            name="b0x", bufs=1
        ) as actx, tc.tile_pool(name="b0q", bufs=2) as qkp, tc.tile_pool(
            name="b0v", bufs=1
        ) as vp0:
            X0 = actx.tile([P, ND, S], f16, tag="x0", name="X0")
            nc.sync.dma_start(
                out=X0,
                in_=din["srcT"].ap().rearrange("(j p) s -> p j s", p=P),
            )
            X8 = actx.tile([P, 8, S], f8, tag="x8", name="X8")
            nc.sync.dma_start(
                out=X8[:, 0:ND, :],
                in_=din["srcT8"].ap().rearrange("(j p) s -> p j s", p=P),
            )
            nc.vector.memset(X8[0:1, 6, :], 1.0)
            nc.vector.memset(X8[0:1, 7, :], 0.0)

            V0 = vp0.tile([P, NS // 2, 2, VW], f8, name="V0")

            # block1 q-projection depends only on inputs: emit first to fill
            # the startup bubble while block0 weights stream in.
            q1w = load_w8(din["l1_qT8"], D)
            bq1 = par["l1_bq"]

            def ev_q1(m, ps, q0):
                with nc.allow_low_precision(reason="fp8 evac"):
                    if m % 2 == 0:
                        nc.vector.tensor_scalar_add(
                            Q1[:, m // NP, m % NP, :], ps, bq1[:, m : m + 1]
                        )
                    else:
                        nc.scalar.activation(
                            Q1[:, m // NP, m % NP, :], ps, AF.Identity,
                            bias=bq1[:, m : m + 1],
                        )

            fm_proj8(S1T8, q1w, SH, ev_q1)

            # --- q/k/v projections (fp8 DR) ---
            Qt0 = qkp.tile([P, 2, NP, S], f8, tag="qk", name="Qt0")
            Kt0 = qkp.tile([P, 2, NP, S], f8, tag="qk", name="Kt0")
            kw = load_w8(din["l0_kT8"], D)
            bk = par["l0_bk"]

            def ev_k(m, ps, q0):
                with nc.allow_low_precision(reason="fp8 evac"):
                    if m % 2 == 0:
                        nc.vector.tensor_scalar_add(
                            Kt0[:, m // NP, m % NP, :], ps, bk[:, m : m + 1]
                        )
                    else:
                        nc.scalar.activation(
                            Kt0[:, m // NP, m % NP, :], ps, AF.Identity,
                            bias=bk[:, m : m + 1],
                        )

            fm_proj8(X8, kw, S, ev_k)

            qw = load_w8(din["l0_qT8"], D)
            bq = par["l0_bq"]

            def ev_q(m, ps, q0):
                with nc.allow_low_precision(reason="fp8 evac"):
                    if m % 2 == 0:
                        nc.vector.tensor_scalar_add(
                            Qt0[:, m // NP, m % NP, :], ps, bq[:, m : m + 1]
                        )
                    else:
                        nc.scalar.activation(
                            Qt0[:, m // NP, m % NP, :], ps, AF.Identity,
                            bias=bq[:, m : m + 1],
                        )

            fm_proj8(X8, qw, S, ev_q)

            vw = load_w8(din["l0_vT8"], VW)
            vb8 = wp8.tile([1, 2, VW], f8, tag="w8", name="vb0")
            nc.sync.dma_start(
                out=vb8,
                in_=din["l0_vT8"].ap()[D : D + 2, :].rearrange(
                    "(two p) w -> p two w", two=2
                ),
            )
            v_proj8(X8, vw, vb8, V0)

            # prefetch all block0 fp16 weights during attention
            ow = load_w(din["l0_oT"], ND)
            w1 = load_w(din["l0_w1T"], ND)
            w2 = load_w(din["l0_w2T"], ND)

            CTX0 = act6.tile([P, ND, S], f16, tag="a6", name="CTX0")
            Z0a = act6.tile([P, ND, S], f16, tag="a6", name="Z0a")
            ATT0 = act6.tile([P, ND, S], f16, tag="a6", name="ATT0")
            H10 = act6.tile([P, ND, S], f16, tag="a6", name="H10")
            Z0b = act6.tile([P, ND, S], f16, tag="a6", name="Z0b")

            bo = par["l0_bo"]
            ag, ab = par["l0_ag"], par["l0_ab"]
            b1 = par["l0_b1"]
            b2 = par["l0_b2"]
            fg, fb = par["l0_fg"], par["l0_fb"]

            def fm_proj(x_k, w_tiles, q0, Sx, evac, ms=None):
                for m in ms if ms is not None else range(ND):
                    ps = psA.tile([P, Sx], f32, tag="psA", name=f"ps_m{m}")
                    for c in range(Sx // 512):
                        sl = slice(q0 + 512 * c, q0 + 512 * (c + 1))
                        psl = slice(512 * c, 512 * (c + 1))
                        for ki, (xk, wk) in enumerate(zip(x_k, w_tiles)):
                            nc.tensor.matmul(
                                ps[:, psl],
                                wk[:, m * P : (m + 1) * P],
                                xk[:, sl],
                                start=(ki == 0),
                                stop=(ki == len(w_tiles) - 1),
                            )
                    evac(m, ps, q0)

            ctx0_k = [CTX0[:, k, :] for k in range(ND)]
            att0_k = [ATT0[:, k, :] for k in range(ND)]
            h10_k = [H10[:, k, :] for k in range(ND)]

            def ev_o(m, ps, q0):
                nc.vector.scalar_tensor_tensor(
                    Z0a[:, m, q0 : q0 + 512], ps, bo[:, m : m + 1],
                    X0[:, m, q0 : q0 + 512], OP.add, OP.add,
                )

            def out_att(k, sl, t1):
                nc.vector.tensor_scalar(
                    ATT0[:, k, sl], t1, ag[:, k : k + 1], ab[:, k : k + 1],
                    OP.mult, OP.add,
                )

            def ev_w1(m, ps, q0):
                nc.scalar.activation(
                    H10[:, m, q0 : q0 + 512], ps, AF.Gelu, bias=b1[:, m : m + 1]
                )

            def ev_w2(m, ps, q0):
                nc.vector.scalar_tensor_tensor(
                    Z0b[:, m, q0 : q0 + 512], ps, b2[:, m : m + 1],
                    ATT0[:, m, q0 : q0 + 512], OP.add, OP.add,
                )

            def out_feats(k, sl, t1):
                nc.vector.tensor_scalar(
                    FEATS[:, k, sl], t1, fg[:, k : k + 1], fb[:, k : k + 1],
                    OP.mult, OP.add,
                )
                with nc.allow_low_precision(reason="fp8 feats"):
                    nc.gpsimd.tensor_scalar(
                        F8[:, k, sl], t1, fg[:, k : k + 1], fb[:, k : k + 1],
                        OP.mult, OP.add,
                    )

            def chain_pieces(q0):
                """12 emit-thunks for the post-attention chain on one chunk."""
                pieces = []
                for m in range(ND):
                    pieces.append(
                        lambda m=m: fm_proj(ctx0_k, ow, q0, 512, ev_o, ms=[m])
                    )
                pieces.append(lambda: layernorm(Z0a, q0, 512, out_att))
                pieces.append(
                    lambda: fm_proj(att0_k, w1, q0, 512, ev_w1, ms=[0, 1, 2])
                )
                pieces.append(
                    lambda: fm_proj(att0_k, w1, q0, 512, ev_w1, ms=[3, 4, 5])
                )
                pieces.append(
                    lambda: fm_proj(h10_k, w2, q0, 512, ev_w2, ms=[0, 1, 2])
                )
                pieces.append(
                    lambda: fm_proj(h10_k, w2, q0, 512, ev_w2, ms=[3, 4, 5])
                )
                pieces.append(lambda: layernorm(Z0b, q0, 512, out_feats))
                return pieces

            attention8(0, Qt0, Kt0, V0, CTX0)
            attention8(512, Qt0, Kt0, V0, CTX0, interleave=chain_pieces(0))
            for piece in chain_pieces(512):
                piece()

        # ================= BLOCK 1 (half S on q-side, cross-attention) ======
        with tc.tile_pool(name="b1a", bufs=4) as a6h, tc.tile_pool(
            name="b1b", bufs=1
        ) as a6f, tc.tile_pool(name="b1v", bufs=1) as vp1:
            K1 = a6f.tile([P, 2, NP, S], f8, tag="af", name="K1")
            CTX1 = a6h.tile([P, ND, SH], f16, tag="ah", name="CTX1")
            Z1a = a6h.tile([P, ND, SH], f16, tag="ah", name="Z1a")
            ATT1 = a6h.tile([P, ND, SH], f16, tag="ah", name="ATT1")
            H11 = a6h.tile([P, ND, SH], f16, tag="ah", name="H11")
            Z1b = a6h.tile([P, ND, SH], f16, tag="ah", name="Z1b")
            V1 = vp1.tile([P, NS // 2, 2, VW], f8, name="V1")

            kw1 = load_w8(din["l1_kT8"], D)
            bk1 = par["l1_bk"]

            def ev_k1(m, ps, q0):
                with nc.allow_low_precision(reason="fp8 evac"):
                    if m % 2 == 0:
                        nc.vector.tensor_scalar_add(
                            K1[:, m // NP, m % NP, :], ps, bk1[:, m : m + 1]
                        )
                    else:
                        nc.scalar.activation(
                            K1[:, m // NP, m % NP, :], ps, AF.Identity,
                            bias=bk1[:, m : m + 1],
                        )

            fm_proj8(F8, kw1, S, ev_k1)

            vw1 = load_w8(din["l1_vT8"], VW)
            vb18 = wp8.tile([1, 2, VW], f8, tag="w8", name="vb1")
            nc.sync.dma_start(
                out=vb18,
                in_=din["l1_vT8"].ap()[D : D + 2, :].rearrange(
                    "(two p) w -> p two w", two=2
                ),
            )
            v_proj8(F8, vw1, vb18, V1)

            ow1 = load_w(din["l1_oT"], ND)
            w11 = load_w(din["l1_w1T"], ND)
            w21 = load_w(din["l1_w2T"], ND)

            bo1 = par["l1_bo"]
            ag1, ab1 = par["l1_ag"], par["l1_ab"]
            b11 = par["l1_b1"]
            b21 = par["l1_b2"]
            fg1, fb1 = par["l1_fg"], par["l1_fb"]

            def fm_projh(x_k, w_tiles, evac, ms=None):
                for m in ms if ms is not None else range(ND):
                    ps = psA.tile([P, SH], f32, tag="psA", name=f"psh_m{m}")
                    for ki, (xk, wk) in enumerate(zip(x_k, w_tiles)):
                        nc.tensor.matmul(
                            ps,
                            wk[:, m * P : (m + 1) * P],
                            xk,
                            start=(ki == 0),
                            stop=(ki == len(w_tiles) - 1),
                        )
                    evac(m, ps, 0)

            ctx1_k = [CTX1[:, k, :] for k in range(ND)]
            att1_k = [ATT1[:, k, :] for k in range(ND)]
            h11_k = [H11[:, k, :] for k in range(ND)]

            def ev_o1(m, ps, q0):
                nc.vector.scalar_tensor_tensor(
                    Z1a[:, m, :], ps, bo1[:, m : m + 1], S1T[:, m, :],
                    OP.add, OP.add,
                )

            def out_att1(k, sl, t1):
                nc.vector.tensor_scalar(
                    ATT1[:, k, sl], t1, ag1[:, k : k + 1], ab1[:, k : k + 1],
                    OP.mult, OP.add,
                )

            def ev_w11(m, ps, q0):
                nc.scalar.activation(
                    H11[:, m, :], ps, AF.Gelu, bias=b11[:, m : m + 1]
                )

            def ev_w21(m, ps, q0):
                nc.vector.scalar_tensor_tensor(
                    Z1b[:, m, :], ps, b21[:, m : m + 1], ATT1[:, m, :],
                    OP.add, OP.add,
                )

            def out_ctx1(k, sl, t1):
                nc.vector.tensor_scalar(
                    CTX1p[:, k, sl], t1, fg1[:, k : k + 1], fb1[:, k : k + 1],
                    OP.mult, OP.add,
                )

            pieces1 = []
            for m in range(ND):
                pieces1.append(
                    lambda m=m: fm_projh(ctx1_k, ow1, ev_o1, ms=[m])
                )
            pieces1.append(lambda: layernorm(Z1a, 0, SH, out_att1))
            pieces1.append(lambda: fm_projh(att1_k, w11, ev_w11, ms=[0, 1, 2]))
            pieces1.append(lambda: fm_projh(att1_k, w11, ev_w11, ms=[3, 4, 5]))
            pieces1.append(lambda: fm_projh(h11_k, w21, ev_w21, ms=[0, 1, 2]))
            pieces1.append(lambda: fm_projh(h11_k, w21, ev_w21, ms=[3, 4, 5]))
            pieces1.append(lambda: layernorm(Z1b, 0, SH, out_ctx1))

            attention8(0, Q1, K1, V1, CTX1)
            for piece in pieces1:
                piece()

        # ================= POOL + FINAL =====================================
        with tc.tile_pool(name="late", bufs=2) as lp, tc.tile_pool(
            name="wbig", bufs=14
        ) as wb:
            # weight prefetch first: overlaps the whole pool chain
            pw_t = []
            for k in range(NS):
                t = wb.tile([P, S], f16, tag="wb", name=f"pw{k}")
                nc.sync.dma_start(out=t, in_=din["pwT"].ap()[k * P : (k + 1) * P, :])
                pw_t.append(t)
            pwb = wb.tile([1, S], f16, tag="wb", name="pwb")
            nc.sync.dma_start(out=pwb, in_=din["pwT"].ap()[S : S + 1, :])
            fin_t = []
            for k in range(2 * ND):
                t = wb.tile([P, D], f16, tag="wb", name=f"fin{k}")
                nc.sync.dma_start(
                    out=t, in_=din["finT"].ap()[k * P : (k + 1) * P, :]
                )
                fin_t.append(t)

            # 1) transpose CTX1p [768, 512] -> row-major [512, 768]
            C1RM = lp.tile([P, SH // P, D], f16, tag="lt", name="C1RM")
            for st in range(SH // P):
                tp = psA.tile([P, D], f16, tag="psA", name=f"t1ps{st}")
                for j in range(ND):
                    nc.tensor.transpose(
                        tp[:, j * P : (j + 1) * P],
                        CTX1p[:, j, st * P : (st + 1) * P],
                        ident,
                    )
                nc.vector.tensor_copy(C1RM[:, st, :], tp)
                nc.sync.dma_start(
                    out=scr1.ap().rearrange("(s d) -> s d", d=D)[
                        st * P : (st + 1) * P, :
                    ],
                    in_=C1RM[:, st, :],
                )
            # 2) read back as M_view rows [384, 1024], transpose to [1024, 384]
            MV = lp.tile([P, 3, S], f16, tag="lt", name="MV")
            v2 = scr1.ap().rearrange("(r c) -> r c", c=S)
            for rt in range(3):
                nc.sync.dma_start(out=MV[:, rt, :], in_=v2[rt * P : (rt + 1) * P, :])
            MVT = lp.tile([P, NS + 1, 3 * P], f16, tag="lt", name="MVT")
            nc.vector.memset(MVT[0:1, NS, :], 1.0)
            for ct in range(NS):
                tp = psS.tile([P, 3 * P], f16, tag="psS", name=f"t2ps{ct}")
                for rt in range(3):
                    nc.tensor.transpose(
                        tp[:, rt * P : (rt + 1) * P],
                        MV[:, rt, ct * P : (ct + 1) * P],
                        ident,
                    )
                nc.vector.tensor_copy(MVT[:, ct, :], tp)
            # 3) pool matmul: out_rm [384, 1024] = M_view @ pw.T + pb
            PRM = lp.tile([P, 3, S], f16, tag="lt", name="PRM")
            for rt in range(3):
                ps = psA.tile([P, S], f32, tag="psA", name=f"plps{rt}")
                for c in range(2):
                    sl = slice(512 * c, 512 * (c + 1))
                    for ki in range(NS + 1):
                        if ki < NS:
                            lhs = MVT[:, ki, rt * P : (rt + 1) * P]
                            rhs = pw_t[ki][:, sl]
                        else:
                            lhs = MVT[0:1, NS, rt * P : (rt + 1) * P]
                            rhs = pwb[:, sl]
                        nc.tensor.matmul(
                            ps[:, sl], lhs, rhs, start=(ki == 0), stop=(ki == NS)
                        )
                nc.vector.tensor_copy(PRM[:, rt, :], ps)
                nc.sync.dma_start(
                    out=scr2.ap().rearrange("(r c) -> r c", c=S)[
                        rt * P : (rt + 1) * P, :
                    ],
                    in_=PRM[:, rt, :],
                )
            # 4) read back as app row-major [512, 768], transpose -> APPT'
            APPRM = lp.tile([P, SH // P, D], f16, tag="lt", name="APPRM")
            v3 = scr2.ap().rearrange("(s d) -> s d", d=D)
            for st in range(SH // P):
                nc.sync.dma_start(
                    out=APPRM[:, st, :], in_=v3[st * P : (st + 1) * P, :]
                )
            APPT = lp.tile([P, ND, SH], f16, tag="lt", name="APPT")
            for j in range(ND):
                tp = psS.tile([P, SH], f16, tag="psS", name=f"t3ps{j}")
                for st in range(SH // P):
                    nc.tensor.transpose(
                        tp[:, st * P : (st + 1) * P],
                        APPRM[:, st, j * P : (j + 1) * P],
                        ident,
                    )
                nc.vector.tensor_copy(APPT[:, j, :], tp)
            # 5) final: out' = finT.T @ [feats_half ; app]
            OUTT = lp.tile([P, ND, SH], f32, tag="lt", name="OUTT")
            for m in range(ND):
                ps = psS.tile([P, SH], f32, tag="psS", name=f"fps{m}")
                for ki in range(2 * ND):
                    rhs = (
                        FEATS[:, ki, 0:SH]
                        if ki < ND
                        else APPT[:, ki - ND, :]
                    )
                    nc.tensor.matmul(
                        ps,
                        fin_t[ki][:, m * P : (m + 1) * P],
                        rhs,
                        start=(ki == 0),
                        stop=(ki == 2 * ND - 1),
                    )
                nc.scalar.activation(
                    OUTT[:, m, :], ps, AF.Identity, bias=finb[:, m : m + 1]
                )
            nc.sync.dma_start(
                out=outT.ap().rearrange("(j p) s -> p j s", p=P), in_=OUTT
            )


def _qk_perm():
    """New feature index for each original (h, dh): head h's 64 dims become
    two 32-row planes at partitions 32*(h%4) and d-tiles h//4, h//4+3."""
    perm = np.empty(D, dtype=np.int64)
    for h in range(H):
        for dh in range(DH):
            j = (h // 4) + NP * (dh // 32)
            p = 32 * (h % 4) + (dh % 32)
            perm[h * DH + dh] = j * P + p
    return perm


def _prep_inputs(inputs):
    import ml_dtypes

    f8 = ml_dtypes.float8_e4m3

    e = np.ascontiguousarray(np.asarray(inputs["e"], dtype=np.float32))
    f = np.ascontiguousarray(np.asarray(inputs["f"], dtype=np.float32))
    wq = np.asarray(inputs["wq"], np.float32)
    wk = np.asarray(inputs["wk"], np.float32)
    wv = np.asarray(inputs["wv"], np.float32)
    wo = np.asarray(inputs["wo"], np.float32)
    bq = np.asarray(inputs["bq"], np.float32)
    bk = np.asarray(inputs["bk"], np.float32)
    bv = np.asarray(inputs["bv"], np.float32)
    bo = np.asarray(inputs["bo"], np.float32)
    ag = np.asarray(inputs["attn_ln_g"], np.float32)
    ab = np.asarray(inputs["attn_ln_b"], np.float32)
    w1 = np.asarray(inputs["ffn_w1"], np.float32)
    b1 = np.asarray(inputs["ffn_b1"], np.float32)
    w2 = np.asarray(inputs["ffn_w2"], np.float32)
    b2 = np.asarray(inputs["ffn_b2"], np.float32)
    fg = np.asarray(inputs["ffn_ln_g"], np.float32)
    fb = np.asarray(inputs["ffn_ln_b"], np.float32)
    pw = np.asarray(inputs["pool_w"], np.float32)
    pb = np.asarray(inputs["pool_b"], np.float32)
    fw = np.asarray(inputs["final_w"], np.float32)
    fnb = np.asarray(inputs["final_b"], np.float32)

    perm = _qk_perm()

    def vec6(v):
        return np.ascontiguousarray(v.reshape(ND, P).T)

    def q8(x):
        return np.ascontiguousarray(x).astype(f8).view(np.uint8)

    in_maps = []
    for c in range(8):
        ti, b, h = c // 4, (c // 2) % 2, c % 2
        src = e if ti == 0 else f
        s1 = f if ti == 0 else e
        own = slice(SH * h, SH * (h + 1))
        oth = slice(SH * (1 - h), SH * (2 - h))
        src_b = src[:, b, :]
        src_perm = np.concatenate([src_b[own], src_b[oth]], axis=0)
        srcT = np.ascontiguousarray(src_perm.T)
        s1Tm = np.ascontiguousarray(s1[own, b, :].T)
        m = {
            "srcT": srcT.astype(np.float16),
            "srcT8": q8(srcT),
            "s1T": s1Tm.astype(np.float16),
            "s1T8": q8(s1Tm),
            "pwT": np.ascontiguousarray(
                np.concatenate([pw[ti].T, pb[ti][None, :]], axis=0)
            ).astype(np.float16),
            "finT": np.ascontiguousarray(fw[ti].T).astype(np.float16),
        }
        spar_list = []
        for li in (0, 1):
            # q/k: transpose, scale by WS, permute output features
            qT = wq[ti, li].T * WS
            kT = wk[ti, li].T * WS
            qTp = np.empty_like(qT)
            qTp[:, perm] = qT
            kTp = np.empty_like(kT)
            kTp[:, perm] = kT
            bqp = np.empty(D, np.float32)
            bqp[perm] = bq[ti, li] * WS
            bkp = np.empty(D, np.float32)
            bkp[perm] = bk[ti, li] * WS
            # v: plain transpose + scale, bias row + zero row; cols [D:D+64]
            # are a ones-block (weights 0, bias 1) producing softmax denoms
            vT8 = np.zeros((D + 2, D + DH), np.float32)
            vT8[0:D, 0:D] = wv[ti, li].T * WS
            vT8[D, 0:D] = bv[ti, li] * WS
            vT8[D, D:] = 1.0
            m.update(
                {
                    f"l{li}_qT8": q8(qTp),
                    f"l{li}_kT8": q8(kTp),
                    f"l{li}_vT8": q8(vT8),
                    f"l{li}_oT": np.ascontiguousarray(
                        wo[ti, li].T * (1.0 / WS)
                    ).astype(np.float16),
                    f"l{li}_w1T": np.ascontiguousarray(w1[ti, li].T).astype(np.float16),
                    f"l{li}_w2T": np.ascontiguousarray(w2[ti, li].T).astype(np.float16),
                }
            )
            spar_list.extend([
                vec6(bqp), vec6(bkp), vec6(bo[ti, li]), vec6(b1[ti, li]),
                vec6(b2[ti, li]), vec6(ag[ti, li]), vec6(ab[ti, li]),
                vec6(fg[ti, li]), vec6(fb[ti, li]),
            ])
        spar_list.append(vec6(fnb[ti]))
        m["spar"] = np.ascontiguousarray(
            np.stack(spar_list, axis=1), dtype=np.float32
        )
        in_maps.append(m)
    return in_maps


def get_program():
    if "nc" not in _BUILT:
        _BUILT["nc"] = _build_program()
    return _BUILT["nc"]


def kernel(**inputs):
    from concourse.bass_utils import run_bass_kernel_spmd

    nc = get_program()
    in_maps = _prep_inputs(inputs)
    res = run_bass_kernel_spmd(nc, in_maps, core_ids=list(range(8)))
    c_e_f = np.empty((S, B, D), np.float32)
    c_f_e = np.empty((S, B, D), np.float32)
    for c in range(8):
        ti, b, h = c // 4, (c // 2) % 2, c % 2
        dst = c_e_f if ti == 0 else c_f_e
        dst[SH * h : SH * (h + 1), b, :] = res.results[c]["outT"].T
    return c_e_f, c_f_e


# revision 31
# speedup vs baseline: 1.0155x; 1.0155x over previous
# Trainium2 Bass kernel for nn_Cross_Transformer (dense_transformer).
#
# Sharding: 8 cores = 2 towers x 2 batches x 2 sequence-halves.
# Each core computes block0 (self-attention) in full (its inputs are permuted
# so its own half leads, keeping the program SPMD-uniform), then its half of
# block1 (cross-attention), pool, and final projection. No collectives.
#
# Layout: activations are feature-major [D on partitions, S on free].
# Q/K/V projections, attention scores and attn*V run in fp8e4m3 DoubleRow
# (2 contraction planes per instruction at 0.5 cycles/row). Q/K/V weights are
# pre-scaled by 32 to land in fp8's normal range; the 1/(32*32) shows up in
# the softmax exp scale and 1/32 is folded into the o-projection weights.
# Q/K features are permuted (prep-side) so each head's 64 dims sit as two
# 32-partition planes at free-stride 3*S, making score matmuls DoubleRow-able.
# o-proj, FFN, pool and final matmuls stay fp16 for accuracy.
# LayerNorm over D uses ones-column matmuls (values pre-scaled 1/D, 1/(D-1))
# and a short Square/Rsqrt chain; softmax denominators come from fp8 ones
# DoubleRow matmuls into psum rows 0/32, reciprocals stay on those rows, and
# per-head normalization is broadcast via K=1 matmuls into partition halves.

import math

import numpy as np

S = 1024
B = 2
D = 768
H = 12
DH = 64
EPS = 1e-6
SH = S // 2  # 512, per-core block1 rows
P = 128
ND = D // P  # 6 d-tiles
NS = S // P  # 8 s-tiles
NP = 3  # DoubleRow pair tiles per D-contraction
VW = D + DH  # V width: 12 head blocks + 64-wide ones block (denominators)
WS = 32.0  # fp8 weight pre-scale for q/k/v

_BUILT = {}


def _build_program():
    import concourse.bacc as bacc
    import concourse.tile as tile
    from concourse import mybir
    from concourse.masks import make_identity

    dt = mybir.dt
    f32 = dt.float32
    f16 = dt.float16
    f8 = dt.float8e4
    AF = mybir.ActivationFunctionType
    OP = mybir.AluOpType

    nc = bacc.Bacc("TRN2", target_bir_lowering=False, debug=False, num_devices=8)

    din = {}

    def dram_in(name, shape, dty):
        din[name] = nc.dram_tensor(name, list(shape), dty, kind="ExternalInput")
        return din[name]

    dram_in("srcT", (D, S), f16)
    dram_in("srcT8", (D, S), f8)
    dram_in("s1T", (D, SH), f16)
    dram_in("s1T8", (D, SH), f8)
    for li in (0, 1):
        dram_in(f"l{li}_qT8", (D, D), f8)
        dram_in(f"l{li}_kT8", (D, D), f8)
        dram_in(f"l{li}_vT8", (D + 2, VW), f8)
        dram_in(f"l{li}_oT", (D, D), f16)
        dram_in(f"l{li}_w1T", (D, D), f16)
        dram_in(f"l{li}_w2T", (D, D), f16)
    dram_in("spar", (P, 19, ND), f32)
    dram_in("pwT", (S + 1, S), f16)
    dram_in("finT", (2 * D, D), f16)

    outT = nc.dram_tensor("outT", [D, SH], f32, kind="ExternalOutput")
    scr1 = nc.dram_tensor("scr1", [SH * D], f16, kind="Internal")
    scr2 = nc.dram_tensor("scr2", [SH * D], f16, kind="Internal")

    with tile.TileContext(nc) as tc:
        _emit(nc, tc, tile, dt, AF, OP, din, outT, scr1, scr2, make_identity)

    nc.compile()
    return nc


def _emit(nc, tc, tile, dt, AF, OP, din, outT, scr1, scr2, make_identity):
    f32 = dt.float32
    f16 = dt.float16
    f8 = dt.float8e4
    import contextlib

    DR = None
    from concourse import mybir

    DR = mybir.MatmulPerfMode.DoubleRow
    EXS = 0.125 / (WS * WS)  # softmax exp input scale

    es = contextlib.ExitStack()
    with es:
        persist = es.enter_context(tc.tile_pool(name="persist", bufs=1))
        wp = es.enter_context(tc.tile_pool(name="wp", bufs=18))
        wp8 = es.enter_context(tc.tile_pool(name="wp8", bufs=7))
        psA = es.enter_context(tc.tile_pool(name="psA", bufs=2, space="PSUM"))
        psC = es.enter_context(tc.tile_pool(name="psC", bufs=1, space="PSUM"))
        psD = es.enter_context(tc.tile_pool(name="psD", bufs=1, space="PSUM"))
        psS = es.enter_context(tc.tile_pool(name="psS", bufs=2, space="PSUM"))
        expool = es.enter_context(tc.tile_pool(name="expool", bufs=5))
        sqp = es.enter_context(tc.tile_pool(name="sqp", bufs=4))
        rbp = es.enter_context(tc.tile_pool(name="rbp", bufs=4))
        rsp = es.enter_context(tc.tile_pool(name="rsp", bufs=4))

        # --- constants ---
        ident = persist.tile([P, P], f16, name="ident")
        make_identity(nc, ident)
        ones_mean = persist.tile([P, 1], f16, name="ones_mean")
        nc.vector.memset(ones_mean, 1.0 / D)
        ones_var = persist.tile([P, 1], f16, name="ones_var")
        nc.vector.memset(ones_var, 1.0 / (D - 1))
        ones_r128 = persist.tile([1, P], f16, name="ones_r128")
        nc.vector.memset(ones_r128, 1.0)

        # --- small params (biases, LN): one DMA, sliced views ---
        SPAR = persist.tile([P, 19, ND], f32, name="SPAR")
        nc.sync.dma_start(out=SPAR, in_=din["spar"].ap())
        par = {}
        pnames = ("bq", "bk", "bo", "b1", "b2", "ag", "ab", "fg", "fb")
        for li in (0, 1):
            for bi, bn in enumerate(pnames):
                par[f"l{li}_{bn}"] = SPAR[:, 9 * li + bi, :]
        finb = SPAR[:, 18, :]

        # persistent activations
        FEATS = persist.tile([P, ND, S], f16, name="FEATS")
        F8 = persist.tile([P, 8, S], f8, name="F8")
        nc.vector.memset(F8[0:1, 6, :], 1.0)
        nc.vector.memset(F8[0:1, 7, :], 0.0)
        S1T = persist.tile([P, ND, SH], f16, name="S1T")
        nc.sync.dma_start(
            out=S1T, in_=din["s1T"].ap().rearrange("(j p) s -> p j s", p=P)
        )
        S1T8 = persist.tile([P, ND, SH], f8, name="S1T8")
        nc.sync.dma_start(
            out=S1T8, in_=din["s1T8"].ap().rearrange("(j p) s -> p j s", p=P)
        )
        CTX1p = persist.tile([P, ND, SH], f16, name="CTX1p")
        Q1 = persist.tile([P, 2, NP, SH], f8, name="Q1")
        rb_t = persist.tile([P, S], f16, name="rb_t")
        mrb_t = persist.tile([P, S], f16, name="mrb_t")
        # LN small stats rows
        lnt = persist.tile([1, S], f32, name="lnt")
        lnv = persist.tile([1, S], f32, name="lnv")
        lnr16 = persist.tile([1, S], f16, name="lnr16")
        lnmr16 = persist.tile([1, S], f16, name="lnmr16")

        def load_w(dram_h, nk):
            """fp16 weight k-tiles [P, D]."""
            ap = dram_h.ap()
            tiles = []
            for t in range(nk):
                wt = wp.tile([P, D], f16, tag="w", name=f"w_{dram_h.name}_{t}")
                nc.sync.dma_start(out=wt, in_=ap[t * P : (t + 1) * P, :])
                tiles.append(wt)
            return tiles

        def load_w8(dram_h, width):
            """fp8 DoubleRow pair tiles [P, 2, width]."""
            ap = dram_h.ap()
            tiles = []
            for t in range(NP):
                wt = wp8.tile([P, 2, width], f8, tag="w8", name=f"w8_{dram_h.name}_{t}")
                nc.gpsimd.dma_start(
                    out=wt,
                    in_=ap[2 * t * P : (2 * t + 2) * P, :].rearrange(
                        "(two p) w -> p two w", two=2
                    ),
                )
                tiles.append(wt)
            return tiles

        def fm_proj8(x8, w8s, Sx, evac):
            """fp8 DR projection: out[m] = sum_t w8s[t][:, :, mslice].T (x) x8-pairs."""
            nch = Sx // 512
            for m in range(ND):
                ps = psA.tile([P, Sx], f32, tag="psA", name=f"ps8_m{m}")
                for c in range(nch):
                    sl = slice(512 * c, 512 * (c + 1))
                    for t in range(NP):
                        nc.tensor.matmul(
                            ps[:, sl],
                            w8s[t][:, :, m * P : (m + 1) * P],
                            x8[:, 2 * t : 2 * t + 2, sl],
                            start=(t == 0),
                            stop=(t == NP - 1),
                            perf_mode=DR,
                        )
                evac(m, ps, 0)

        def v_proj8(x8, vw8, vb8, V8):
            """fp8 DR V projection (row-major out, bias via ones/zero planes)."""
            for st in range(NS):
                ps = psA.tile([P, VW], f32, tag="psA", name=f"vps{st}")
                ssl = slice(st * P, (st + 1) * P)
                for c0, c1 in ((0, 512), (512, VW)):
                    for t in range(NP):
                        nc.tensor.matmul(
                            ps[:, c0:c1],
                            x8[:, 2 * t : 2 * t + 2, ssl],
                            vw8[t][:, :, c0:c1],
                            start=(t == 0),
                            stop=False,
                            perf_mode=DR,
                        )
                    nc.tensor.matmul(
                        ps[:, c0:c1],
                        x8[0:1, 6:8, ssl],
                        vb8[:, :, c0:c1],
                        start=False,
                        stop=True,
                        perf_mode=DR,
                    )
                with nc.allow_low_precision(reason="fp8 V"):
                    nc.vector.tensor_copy(V8[:, st // 2, st % 2, :], ps)

        def attention8(q0, Qt, Kt, V8, CTXdst, interleave=(), SQ=512):
            """Per-head attention over q-columns [q0, q0+SQ), fp8 DoubleRow.
            Head h -> CTXdst partitions 64*(h%2), d-tile h//2. Softmax
            denominators come from the shared ones-block in V8."""
            qsl = slice(q0, q0 + SQ)
            for h in range(H):
                hp, hb = h // 2, 64 * (h % 2)
                pb = 32 * (h % 4)
                j0 = h // 4
                ctx_ps = psC.tile([DH, SQ], f32, tag="psC", name=f"ctx{h}")
                den = None
                for t in range(NS // 2):
                    ex = expool.tile([P, 2, SQ], f8, tag="ex", name=f"ex{h}_{t}")
                    sc = psA.tile([P, 2 * SQ], f32, tag="psA", name=f"sc{h}_{t}")
                    for half in (0, 1):
                        st = 2 * t + half
                        nc.tensor.matmul(
                            sc[:, half * SQ : (half + 1) * SQ],
                            Kt[pb : pb + 32, :, j0, st * P : (st + 1) * P],
                            Qt[pb : pb + 32, :, j0, qsl],
                            start=True,
                            stop=True,
                            perf_mode=DR,
                            tile_position=(pb, 0),
                        )
                    nc.scalar.activation(ex, sc, AF.Exp, scale=EXS)
                    if t == 0:
                        den = psD.tile([DH, SQ], f32, tag="psD", name=f"den{h}")
                    nc.tensor.matmul(
                        ctx_ps,
                        V8[:, t, :, DH * h : DH * (h + 1)],
                        ex,
                        start=(t == 0),
                        stop=(t == NS // 2 - 1),
                        perf_mode=DR,
                    )
                    nc.tensor.matmul(
                        den,
                        V8[:, t, :, D : D + DH],
                        ex,
                        start=(t == 0),
                        stop=(t == NS // 2 - 1),
                        perf_mode=DR,
                    )
                rs = rbp.tile([DH, SQ], f16, tag="rb", name=f"rs{h}")
                with nc.allow_low_precision(reason="softmax recip"):
                    nc.vector.reciprocal(rs, den)
                if h % 2 == 0:
                    with nc.allow_low_precision(reason="ctx norm"):
                        nc.vector.tensor_mul(
                            CTXdst[0:DH, hp, qsl], ctx_ps, rs
                        )
                else:
                    cte = rbp.tile([DH, SQ], f16, tag="rb", name=f"cte{h}")
                    with nc.allow_low_precision(reason="ctx evac"):
                        nc.vector.tensor_copy(cte, ctx_ps)
                        nc.gpsimd.tensor_mul(
                            CTXdst[hb : hb + DH, hp, qsl], cte, rs
                        )
                if h < len(interleave):
                    interleave[h]()

        def layernorm(Zt, q0, Sx, out_fn):
            """LN over partitions(d) of Zt[:, :, q0:q0+Sx] (Bessel std)."""
            cw = min(512, Sx)
            nch = Sx // cw
            for c in range(nch):
                sl = slice(q0 + cw * c, q0 + cw * (c + 1))
                sum_ps = psS.tile([1, cw], f32, tag="psS", name=f"lnsum{c}")
                for k in range(ND):
                    nc.tensor.matmul(
                        sum_ps,
                        ones_mean[:, 0:1],
                        Zt[:, k, sl],
                        start=(k == 0),
                        stop=(k == ND - 1),
                    )
                sq_ps = psS.tile([1, cw], f32, tag="psS", name=f"lnsq{c}")
                for k in range(ND):
                    sq = sqp.tile([P, cw], f16, tag="sq", name=f"sq{k}{c}")
                    nc.gpsimd.tensor_mul(sq, Zt[:, k, sl], Zt[:, k, sl])
                    nc.tensor.matmul(
                        sq_ps,
                        ones_var[:, 0:1],
                        sq,
                        start=(k == 0),
                        stop=(k == ND - 1),
                    )
                # var = E[z^2]*D/(D-1) - mean^2*D/(D-1); r = 1/sqrt(var)
                nc.scalar.activation(
                    lnt[:, sl], sum_ps, AF.Square,
                    scale=math.sqrt(D / (D - 1.0)),
                )
                nc.vector.tensor_sub(lnv[:, sl], sq_ps, lnt[:, sl])
                nc.vector.reciprocal(lnv[:, sl], lnv[:, sl])
                nc.scalar.activation(lnr16[:, sl], lnv[:, sl], AF.Sqrt)
                with nc.allow_low_precision(reason="ln mr fp16"):
                    nc.vector.tensor_mul(lnmr16[:, sl], sum_ps, lnr16[:, sl])
            for c in range(nch):
                sl = slice(q0 + cw * c, q0 + cw * (c + 1))
                rb_ps = psS.tile([P, cw], f32, tag="psS", name=f"rbps{c}")
                nc.tensor.matmul(
                    rb_ps, ones_r128, lnr16[0:1, sl], start=True, stop=True
                )
                nc.vector.tensor_copy(rb_t[:, sl], rb_ps)
                mrb_ps = psS.tile([P, cw], f32, tag="psS", name=f"mrbps{c}")
                nc.tensor.matmul(
                    mrb_ps, ones_r128, lnmr16[0:1, sl], start=True, stop=True
                )
                nc.vector.tensor_copy(mrb_t[:, sl], mrb_ps)
                for k in range(ND):
                    t1 = sqp.tile([P, cw], f16, tag="sq", name=f"ap{k}{c}")
                    nc.vector.tensor_mul(t1, Zt[:, k, sl], rb_t[:, sl])
                    nc.vector.tensor_sub(t1, t1, mrb_t[:, sl])
                    out_fn(k, sl, t1)

        # ================= BLOCK 0 (full S, self-attention on src) =========
        with tc.tile_pool(name="b0a", bufs=5) as act6, tc.tile_pool(
            name="b0x", bufs=1
        ) as actx, tc.tile_pool(name="b0q", bufs=2) as qkp, tc.tile_pool(
            name="b0v", bufs=1
        ) as vp0:
            X0 = actx.tile([P, ND, S], f16, tag="x0", name="X0")
            nc.sync.dma_start(
                out=X0,
                in_=din["srcT"].ap().rearrange("(j p) s -> p j s", p=P),
            )
            X8 = actx.tile([P, 8, S], f8, tag="x8", name="X8")
            nc.sync.dma_start(
                out=X8[:, 0:ND, :],
                in_=din["srcT8"].ap().rearrange("(j p) s -> p j s", p=P),
            )
            nc.vector.memset(X8[0:1, 6, :], 1.0)
            nc.vector.memset(X8[0:1, 7, :], 0.0)

            V0 = vp0.tile([P, NS // 2, 2, VW], f8, name="V0")

            # block1 q-projection depends only on inputs: emit first to fill
            # the startup bubble while block0 weights stream in.
            q1w = load_w8(din["l1_qT8"], D)
            bq1 = par["l1_bq"]

            def ev_q1(m, ps, q0):
                with nc.allow_low_precision(reason="fp8 evac"):
                    if m % 2 == 0:
                        nc.vector.tensor_scalar_add(
                            Q1[:, m // NP, m % NP, :], ps, bq1[:, m : m + 1]
                        )
                    else:
                        nc.scalar.activation(
                            Q1[:, m // NP, m % NP, :], ps, AF.Identity,
                            bias=bq1[:, m : m + 1],
                        )

            fm_proj8(S1T8, q1w, SH, ev_q1)

            # --- q/k/v projections (fp8 DR) ---
            Qt0 = qkp.tile([P, 2, NP, S], f8, tag="qk", name="Qt0")
            Kt0 = qkp.tile([P, 2, NP, S], f8, tag="qk", name="Kt0")
            kw = load_w8(din["l0_kT8"], D)
            bk = par["l0_bk"]

            def ev_k(m, ps, q0):
                with nc.allow_low_precision(reason="fp8 evac"):
                    if m % 2 == 0:
                        nc.vector.tensor_scalar_add(
                            Kt0[:, m // NP, m % NP, :], ps, bk[:, m : m + 1]
                        )
                    else:
                        nc.scalar.activation(
                            Kt0[:, m // NP, m % NP, :], ps, AF.Identity,
                            bias=bk[:, m : m + 1],
                        )

            fm_proj8(X8, kw, S, ev_k)

            qw = load_w8(din["l0_qT8"], D)
            bq = par["l0_bq"]

            def ev_q(m, ps, q0):
                with nc.allow_low_precision(reason="fp8 evac"):
                    if m % 2 == 0:
                        nc.vector.tensor_scalar_add(
                            Qt0[:, m // NP, m % NP, :], ps, bq[:, m : m + 1]
                        )
                    else:
                        nc.scalar.activation(
                            Qt0[:, m // NP, m % NP, :], ps, AF.Identity,
                            bias=bq[:, m : m + 1],
                        )

            fm_proj8(X8, qw, S, ev_q)

            vw = load_w8(din["l0_vT8"], VW)
            vb8 = wp8.tile([1, 2, VW], f8, tag="w8", name="vb0")
            nc.sync.dma_start(
                out=vb8,
                in_=din["l0_vT8"].ap()[D : D + 2, :].rearrange(
                    "(two p) w -> p two w", two=2
                ),
            )
            v_proj8(X8, vw, vb8, V0)

            # prefetch all block0 fp16 weights during attention
            ow = load_w(din["l0_oT"], ND)
            w1 = load_w(din["l0_w1T"], ND)
            w2 = load_w(din["l0_w2T"], ND)

            CTX0 = act6.tile([P, ND, S], f16, tag="a6", name="CTX0")
            Z0a = act6.tile([P, ND, S], f16, tag="a6", name="Z0a")
            ATT0 = act6.tile([P, ND, S], f16, tag="a6", name="ATT0")
            H10 = act6.tile([P, ND, S], f16, tag="a6", name="H10")
            Z0b = act6.tile([P, ND, S], f16, tag="a6", name="Z0b")

            bo = par["l0_bo"]
            ag, ab = par["l0_ag"], par["l0_ab"]
            b1 = par["l0_b1"]
            b2 = par["l0_b2"]
            fg, fb = par["l0_fg"], par["l0_fb"]

            def fm_proj(x_k, w_tiles, q0, Sx, evac, ms=None):
                cw = min(512, Sx)
                for m in ms if ms is not None else range(ND):
                    ps = psA.tile([P, Sx], f32, tag="psA", name=f"ps_m{m}")
                    for c in range(Sx // cw):
                        sl = slice(q0 + cw * c, q0 + cw * (c + 1))
                        psl = slice(cw * c, cw * (c + 1))
                        for ki, (xk, wk) in enumerate(zip(x_k, w_tiles)):
                            nc.tensor.matmul(
                                ps[:, psl],
                                wk[:, m * P : (m + 1) * P],
                                xk[:, sl],
                                start=(ki == 0),
                                stop=(ki == len(w_tiles) - 1),
                            )
                    evac(m, ps, q0)

            ctx0_k = [CTX0[:, k, :] for k in range(ND)]
            att0_k = [ATT0[:, k, :] for k in range(ND)]
            h10_k = [H10[:, k, :] for k in range(ND)]

            def mk_ev_o(w):
                def ev_o(m, ps, q0):
                    nc.vector.scalar_tensor_tensor(
                        Z0a[:, m, q0 : q0 + w], ps, bo[:, m : m + 1],
                        X0[:, m, q0 : q0 + w], OP.add, OP.add,
                    )
                return ev_o

            def out_att(k, sl, t1):
                nc.vector.tensor_scalar(
                    ATT0[:, k, sl], t1, ag[:, k : k + 1], ab[:, k : k + 1],
                    OP.mult, OP.add,
                )

            def mk_ev_w1(w):
                def ev_w1(m, ps, q0):
                    nc.scalar.activation(
                        H10[:, m, q0 : q0 + w], ps, AF.Gelu, bias=b1[:, m : m + 1]
                    )
                return ev_w1

            def mk_ev_w2(w):
                def ev_w2(m, ps, q0):
                    nc.vector.scalar_tensor_tensor(
                        Z0b[:, m, q0 : q0 + w], ps, b2[:, m : m + 1],
                        ATT0[:, m, q0 : q0 + w], OP.add, OP.add,
                    )
                return ev_w2

            def out_feats(k, sl, t1):
                nc.vector.tensor_scalar(
                    FEATS[:, k, sl], t1, fg[:, k : k + 1], fb[:, k : k + 1],
                    OP.mult, OP.add,
                )
                with nc.allow_low_precision(reason="fp8 feats"):
                    nc.gpsimd.tensor_scalar(
                        F8[:, k, sl], t1, fg[:, k : k + 1], fb[:, k : k + 1],
                        OP.mult, OP.add,
                    )

            CH = 512  # block0 pipeline chunk width

            def chain_pieces(q0):
                """12 emit-thunks for the post-attention chain on one chunk."""
                ev_o, ev_w1, ev_w2 = mk_ev_o(CH), mk_ev_w1(CH), mk_ev_w2(CH)
                pieces = []
                for m in range(ND):
                    pieces.append(
                        lambda m=m: fm_proj(ctx0_k, ow, q0, CH, ev_o, ms=[m])
                    )
                pieces.append(lambda: layernorm(Z0a, q0, CH, out_att))
                pieces.append(
                    lambda: fm_proj(att0_k, w1, q0, CH, ev_w1, ms=[0, 1, 2])
                )
                pieces.append(
                    lambda: fm_proj(att0_k, w1, q0, CH, ev_w1, ms=[3, 4, 5])
                )
                pieces.append(
                    lambda: fm_proj(h10_k, w2, q0, CH, ev_w2, ms=[0, 1, 2])
                )
                pieces.append(
                    lambda: fm_proj(h10_k, w2, q0, CH, ev_w2, ms=[3, 4, 5])
                )
                pieces.append(lambda: layernorm(Z0b, q0, CH, out_feats))
                return pieces

            attention8(0, Qt0, Kt0, V0, CTX0, SQ=CH)
            for ci in range(1, S // CH):
                attention8(
                    ci * CH, Qt0, Kt0, V0, CTX0,
                    interleave=chain_pieces((ci - 1) * CH), SQ=CH,
                )
            for piece in chain_pieces(S - CH):
                piece()

        # ================= BLOCK 1 (half S on q-side, cross-attention) ======
        with tc.tile_pool(name="b1a", bufs=4) as a6h, tc.tile_pool(
            name="b1b", bufs=1
        ) as a6f, tc.tile_pool(name="b1v", bufs=1) as vp1:
            K1 = a6f.tile([P, 2, NP, S], f8, tag="af", name="K1")
            CTX1 = a6h.tile([P, ND, SH], f16, tag="ah", name="CTX1")
            Z1a = a6h.tile([P, ND, SH], f16, tag="ah", name="Z1a")
            ATT1 = a6h.tile([P, ND, SH], f16, tag="ah", name="ATT1")
            H11 = a6h.tile([P, ND, SH], f16, tag="ah", name="H11")
            Z1b = a6h.tile([P, ND, SH], f16, tag="ah", name="Z1b")
            V1 = vp1.tile([P, NS // 2, 2, VW], f8, name="V1")

            kw1 = load_w8(din["l1_kT8"], D)
            bk1 = par["l1_bk"]

            def ev_k1(m, ps, q0):
                with nc.allow_low_precision(reason="fp8 evac"):
                    if m % 2 == 0:
                        nc.vector.tensor_scalar_add(
                            K1[:, m // NP, m % NP, :], ps, bk1[:, m : m + 1]
                        )
                    else:
                        nc.scalar.activation(
                            K1[:, m // NP, m % NP, :], ps, AF.Identity,
                            bias=bk1[:, m : m + 1],
                        )

            fm_proj8(F8, kw1, S, ev_k1)

            vw1 = load_w8(din["l1_vT8"], VW)
            vb18 = wp8.tile([1, 2, VW], f8, tag="w8", name="vb1")
            nc.sync.dma_start(
                out=vb18,
                in_=din["l1_vT8"].ap()[D : D + 2, :].rearrange(
                    "(two p) w -> p two w", two=2
                ),
            )
            v_proj8(F8, vw1, vb18, V1)

            ow1 = load_w(din["l1_oT"], ND)
            w11 = load_w(din["l1_w1T"], ND)
            w21 = load_w(din["l1_w2T"], ND)

            bo1 = par["l1_bo"]
            ag1, ab1 = par["l1_ag"], par["l1_ab"]
            b11 = par["l1_b1"]
            b21 = par["l1_b2"]
            fg1, fb1 = par["l1_fg"], par["l1_fb"]

            def fm_projh(x_k, w_tiles, evac, ms=None):
                for m in ms if ms is not None else range(ND):
                    ps = psA.tile([P, SH], f32, tag="psA", name=f"psh_m{m}")
                    for ki, (xk, wk) in enumerate(zip(x_k, w_tiles)):
                        nc.tensor.matmul(
                            ps,
                            wk[:, m * P : (m + 1) * P],
                            xk,
                            start=(ki == 0),
                            stop=(ki == len(w_tiles) - 1),
                        )
                    evac(m, ps, 0)

            ctx1_k = [CTX1[:, k, :] for k in range(ND)]
            att1_k = [ATT1[:, k, :] for k in range(ND)]
            h11_k = [H11[:, k, :] for k in range(ND)]

            def ev_o1(m, ps, q0):
                nc.vector.scalar_tensor_tensor(
                    Z1a[:, m, :], ps, bo1[:, m : m + 1], S1T[:, m, :],
                    OP.add, OP.add,
                )

            def out_att1(k, sl, t1):
                nc.vector.tensor_scalar(
                    ATT1[:, k, sl], t1, ag1[:, k : k + 1], ab1[:, k : k + 1],
                    OP.mult, OP.add,
                )

            def ev_w11(m, ps, q0):
                nc.scalar.activation(
                    H11[:, m, :], ps, AF.Gelu, bias=b11[:, m : m + 1]
                )

            def ev_w21(m, ps, q0):
                nc.vector.scalar_tensor_tensor(
                    Z1b[:, m, :], ps, b21[:, m : m + 1], ATT1[:, m, :],
                    OP.add, OP.add,
                )

            def out_ctx1(k, sl, t1):
                nc.vector.tensor_scalar(
                    CTX1p[:, k, sl], t1, fg1[:, k : k + 1], fb1[:, k : k + 1],
                    OP.mult, OP.add,
                )

            pieces1 = []
            for m in range(ND):
                pieces1.append(
                    lambda m=m: fm_projh(ctx1_k, ow1, ev_o1, ms=[m])
                )
            pieces1.append(lambda: layernorm(Z1a, 0, SH, out_att1))
            pieces1.append(lambda: fm_projh(att1_k, w11, ev_w11, ms=[0, 1, 2]))
            pieces1.append(lambda: fm_projh(att1_k, w11, ev_w11, ms=[3, 4, 5]))
            pieces1.append(lambda: fm_projh(h11_k, w21, ev_w21, ms=[0, 1, 2]))
            pieces1.append(lambda: fm_projh(h11_k, w21, ev_w21, ms=[3, 4, 5]))
            pieces1.append(lambda: layernorm(Z1b, 0, SH, out_ctx1))

            attention8(0, Q1, K1, V1, CTX1)
            for piece in pieces1:
                piece()

        # ================= POOL + FINAL =====================================
        with tc.tile_pool(name="late", bufs=2) as lp, tc.tile_pool(
            name="wbig", bufs=14
        ) as wb:
            # weight prefetch first: overlaps the whole pool chain
            pw_t = []
            for k in range(NS):
                t = wb.tile([P, S], f16, tag="wb", name=f"pw{k}")
                nc.sync.dma_start(out=t, in_=din["pwT"].ap()[k * P : (k + 1) * P, :])
                pw_t.append(t)
            pwb = wb.tile([1, S], f16, tag="wb", name="pwb")
            nc.sync.dma_start(out=pwb, in_=din["pwT"].ap()[S : S + 1, :])
            fin_t = []
            for k in range(2 * ND):
                t = wb.tile([P, D], f16, tag="wb", name=f"fin{k}")
                nc.sync.dma_start(
                    out=t, in_=din["finT"].ap()[k * P : (k + 1) * P, :]
                )
                fin_t.append(t)

            # 1) transpose CTX1p [768, 512] -> row-major [512, 768]
            C1RM = lp.tile([P, SH // P, D], f16, tag="lt", name="C1RM")
            for st in range(SH // P):
                tp = psA.tile([P, D], f16, tag="psA", name=f"t1ps{st}")
                for j in range(ND):
                    nc.tensor.transpose(
                        tp[:, j * P : (j + 1) * P],
                        CTX1p[:, j, st * P : (st + 1) * P],
                        ident,
                    )
                nc.vector.tensor_copy(C1RM[:, st, :], tp)
                nc.sync.dma_start(
                    out=scr1.ap().rearrange("(s d) -> s d", d=D)[
                        st * P : (st + 1) * P, :
                    ],
                    in_=C1RM[:, st, :],
                )
            # 2) read back as M_view rows [384, 1024], transpose to [1024, 384]
            MV = lp.tile([P, 3, S], f16, tag="lt", name="MV")
            v2 = scr1.ap().rearrange("(r c) -> r c", c=S)
            for rt in range(3):
                nc.sync.dma_start(out=MV[:, rt, :], in_=v2[rt * P : (rt + 1) * P, :])
            MVT = lp.tile([P, NS + 1, 3 * P], f16, tag="lt", name="MVT")
            nc.vector.memset(MVT[0:1, NS, :], 1.0)
            for ct in range(NS):
                tp = psS.tile([P, 3 * P], f16, tag="psS", name=f"t2ps{ct}")
                for rt in range(3):
                    nc.tensor.transpose(
                        tp[:, rt * P : (rt + 1) * P],
                        MV[:, rt, ct * P : (ct + 1) * P],
                        ident,
                    )
                nc.vector.tensor_copy(MVT[:, ct, :], tp)
            # 3) pool matmul: out_rm [384, 1024] = M_view @ pw.T + pb
            PRM = lp.tile([P, 3, S], f16, tag="lt", name="PRM")
            for rt in range(3):
                ps = psA.tile([P, S], f32, tag="psA", name=f"plps{rt}")
                for c in range(2):
                    sl = slice(512 * c, 512 * (c + 1))
                    for ki in range(NS + 1):
                        if ki < NS:
                            lhs = MVT[:, ki, rt * P : (rt + 1) * P]
                            rhs = pw_t[ki][:, sl]
                        else:
                            lhs = MVT[0:1, NS, rt * P : (rt + 1) * P]
                            rhs = pwb[:, sl]
                        nc.tensor.matmul(
                            ps[:, sl], lhs, rhs, start=(ki == 0), stop=(ki == NS)
                        )
                nc.vector.tensor_copy(PRM[:, rt, :], ps)
                nc.sync.dma_start(
                    out=scr2.ap().rearrange("(r c) -> r c", c=S)[
                        rt * P : (rt + 1) * P, :
                    ],
                    in_=PRM[:, rt, :],
                )
            # 4) read back as app row-major [512, 768], transpose -> APPT'
            APPRM = lp.tile([P, SH // P, D], f16, tag="lt", name="APPRM")
            v3 = scr2.ap().rearrange("(s d) -> s d", d=D)
            for st in range(SH // P):
                nc.sync.dma_start(
                    out=APPRM[:, st, :], in_=v3[st * P : (st + 1) * P, :]
                )
            APPT = lp.tile([P, ND, SH], f16, tag="lt", name="APPT")
            for j in range(ND):
                tp = psS.tile([P, SH], f16, tag="psS", name=f"t3ps{j}")
                for st in range(SH // P):
                    nc.tensor.transpose(
                        tp[:, st * P : (st + 1) * P],
                        APPRM[:, st, j * P : (j + 1) * P],
                        ident,
                    )
                nc.vector.tensor_copy(APPT[:, j, :], tp)
            # 5) final: out' = finT.T @ [feats_half ; app]
            OUTT = lp.tile([P, ND, SH], f32, tag="lt", name="OUTT")
            for m in range(ND):
                ps = psS.tile([P, SH], f32, tag="psS", name=f"fps{m}")
                for ki in range(2 * ND):
                    rhs = (
                        FEATS[:, ki, 0:SH]
                        if ki < ND
                        else APPT[:, ki - ND, :]
                    )
                    nc.tensor.matmul(
                        ps,
                        fin_t[ki][:, m * P : (m + 1) * P],
                        rhs,
                        start=(ki == 0),
                        stop=(ki == 2 * ND - 1),
                    )
                nc.scalar.activation(
                    OUTT[:, m, :], ps, AF.Identity, bias=finb[:, m : m + 1]
                )
            nc.sync.dma_start(
                out=outT.ap().rearrange("(j p) s -> p j s", p=P), in_=OUTT
            )


def _qk_perm():
    """New feature index for each original (h, dh): head h's 64 dims become
    two 32-row planes at partitions 32*(h%4) and d-tiles h//4, h//4+3."""
    perm = np.empty(D, dtype=np.int64)
    for h in range(H):
        for dh in range(DH):
            j = (h // 4) + NP * (dh // 32)
            p = 32 * (h % 4) + (dh % 32)
            perm[h * DH + dh] = j * P + p
    return perm


def _prep_inputs(inputs):
    import ml_dtypes

    f8 = ml_dtypes.float8_e4m3

    e = np.ascontiguousarray(np.asarray(inputs["e"], dtype=np.float32))
    f = np.ascontiguousarray(np.asarray(inputs["f"], dtype=np.float32))
    wq = np.asarray(inputs["wq"], np.float32)
    wk = np.asarray(inputs["wk"], np.float32)
    wv = np.asarray(inputs["wv"], np.float32)
    wo = np.asarray(inputs["wo"], np.float32)
    bq = np.asarray(inputs["bq"], np.float32)
    bk = np.asarray(inputs["bk"], np.float32)
    bv = np.asarray(inputs["bv"], np.float32)
    bo = np.asarray(inputs["bo"], np.float32)
    ag = np.asarray(inputs["attn_ln_g"], np.float32)
    ab = np.asarray(inputs["attn_ln_b"], np.float32)
    w1 = np.asarray(inputs["ffn_w1"], np.float32)
    b1 = np.asarray(inputs["ffn_b1"], np.float32)
    w2 = np.asarray(inputs["ffn_w2"], np.float32)
    b2 = np.asarray(inputs["ffn_b2"], np.float32)
    fg = np.asarray(inputs["ffn_ln_g"], np.float32)
    fb = np.asarray(inputs["ffn_ln_b"], np.float32)
    pw = np.asarray(inputs["pool_w"], np.float32)
    pb = np.asarray(inputs["pool_b"], np.float32)
    fw = np.asarray(inputs["final_w"], np.float32)
    fnb = np.asarray(inputs["final_b"], np.float32)

    perm = _qk_perm()

    def vec6(v):
        return np.ascontiguousarray(v.reshape(ND, P).T)

    def q8(x):
        return np.ascontiguousarray(x).astype(f8).view(np.uint8)

    in_maps = []
    for c in range(8):
        ti, b, h = c // 4, (c // 2) % 2, c % 2
        src = e if ti == 0 else f
        s1 = f if ti == 0 else e
        own = slice(SH * h, SH * (h + 1))
        oth = slice(SH * (1 - h), SH * (2 - h))
        src_b = src[:, b, :]
        src_perm = np.concatenate([src_b[own], src_b[oth]], axis=0)
        srcT = np.ascontiguousarray(src_perm.T)
        s1Tm = np.ascontiguousarray(s1[own, b, :].T)
        m = {
            "srcT": srcT.astype(np.float16),
            "srcT8": q8(srcT),
            "s1T": s1Tm.astype(np.float16),
            "s1T8": q8(s1Tm),
            "pwT": np.ascontiguousarray(
                np.concatenate([pw[ti].T, pb[ti][None, :]], axis=0)
            ).astype(np.float16),
            "finT": np.ascontiguousarray(fw[ti].T).astype(np.float16),
        }
        spar_list = []
        for li in (0, 1):
            # q/k: transpose, scale by WS, permute output features
            qT = wq[ti, li].T * WS
            kT = wk[ti, li].T * WS
            qTp = np.empty_like(qT)
            qTp[:, perm] = qT
            kTp = np.empty_like(kT)
            kTp[:, perm] = kT
            bqp = np.empty(D, np.float32)
            bqp[perm] = bq[ti, li] * WS
            bkp = np.empty(D, np.float32)
            bkp[perm] = bk[ti, li] * WS
            # v: plain transpose + scale, bias row + zero row; cols [D:D+64]
            # are a ones-block (weights 0, bias 1) producing softmax denoms
            vT8 = np.zeros((D + 2, D + DH), np.float32)
            vT8[0:D, 0:D] = wv[ti, li].T * WS
            vT8[D, 0:D] = bv[ti, li] * WS
            vT8[D, D:] = 1.0
            m.update(
                {
                    f"l{li}_qT8": q8(qTp),
                    f"l{li}_kT8": q8(kTp),
                    f"l{li}_vT8": q8(vT8),
                    f"l{li}_oT": np.ascontiguousarray(
                        wo[ti, li].T * (1.0 / WS)
                    ).astype(np.float16),
                    f"l{li}_w1T": np.ascontiguousarray(w1[ti, li].T).astype(np.float16),
                    f"l{li}_w2T": np.ascontiguousarray(w2[ti, li].T).astype(np.float16),
                }
            )
            spar_list.extend([
                vec6(bqp), vec6(bkp), vec6(bo[ti, li]), vec6(b1[ti, li]),
                vec6(b2[ti, li]), vec6(ag[ti, li]), vec6(ab[ti, li]),
                vec6(fg[ti, li]), vec6(fb[ti, li]),
            ])
        spar_list.append(vec6(fnb[ti]))
        m["spar"] = np.ascontiguousarray(
            np.stack(spar_list, axis=1), dtype=np.float32
        )
        in_maps.append(m)
    return in_maps


def get_program():
    if "nc" not in _BUILT:
        _BUILT["nc"] = _build_program()
    return _BUILT["nc"]


def kernel(**inputs):
    from concourse.bass_utils import run_bass_kernel_spmd

    nc = get_program()
    in_maps = _prep_inputs(inputs)
    res = run_bass_kernel_spmd(nc, in_maps, core_ids=list(range(8)))
    c_e_f = np.empty((S, B, D), np.float32)
    c_f_e = np.empty((S, B, D), np.float32)
    for c in range(8):
        ti, b, h = c // 4, (c // 2) % 2, c % 2
        dst = c_e_f if ti == 0 else c_f_e
        dst[SH * h : SH * (h + 1), b, :] = res.results[c]["outT"].T
    return c_e_f, c_f_e


# revision 43
# speedup vs baseline: 1.1012x; 1.0844x over previous
# Trainium2 Bass kernel for nn_Cross_Transformer (dense_transformer).
#
# Sharding: 8 cores = 2 towers x 2 batches x 2 sequence-halves.
# Each core computes block0 (self-attention) in full (its inputs are permuted
# so its own half leads, keeping the program SPMD-uniform), then its half of
# block1 (cross-attention), pool, and final projection. No collectives.
#
# Layout: activations are feature-major [D on partitions, S on free].
# Q/K/V projections, attention scores and attn*V run in fp8e4m3 DoubleRow
# (2 contraction planes per instruction at 0.5 cycles/row). Q/K/V weights are
# pre-scaled by 32 to land in fp8's normal range; the 1/(32*32) shows up in
# the softmax exp scale and 1/32 is folded into the o-projection weights.
# Q/K features are permuted (prep-side) so each head's 64 dims sit as two
# 32-partition planes at free-stride 3*S, making score matmuls DoubleRow-able.
# o-proj, FFN, pool and final matmuls stay fp16 for accuracy.
# LayerNorm over D uses ones-column matmuls (values pre-scaled 1/D, 1/(D-1))
# and a short Square/Rsqrt chain; softmax denominators come from fp8 ones
# DoubleRow matmuls into psum rows 0/32, reciprocals stay on those rows, and
# per-head normalization is broadcast via K=1 matmuls into partition halves.

import math

import numpy as np

S = 1024
B = 2
D = 768
H = 12
DH = 64
EPS = 1e-6
SH = S // 2  # 512, per-core block1 rows
P = 128
ND = D // P  # 6 d-tiles
NS = S // P  # 8 s-tiles
NP = 3  # DoubleRow pair tiles per D-contraction
VW = D + DH  # V width: 12 head blocks + 64-wide ones block (denominators)
WS = 32.0  # fp8 weight pre-scale for q/k/v

_BUILT = {}


def _build_program():
    import concourse.bacc as bacc
    import concourse.tile as tile
    from concourse import mybir
    from concourse.masks import make_identity

    dt = mybir.dt
    f32 = dt.float32
    f16 = dt.float16
    f8 = dt.float8e4
    AF = mybir.ActivationFunctionType
    OP = mybir.AluOpType

    nc = bacc.Bacc("TRN2", target_bir_lowering=False, debug=False, num_devices=8)

    din = {}

    def dram_in(name, shape, dty):
        din[name] = nc.dram_tensor(name, list(shape), dty, kind="ExternalInput")
        return din[name]

    dram_in("srcT", (D, S), f16)
    dram_in("srcT8", (D, S), f8)
    dram_in("s1T", (D, SH), f16)
    dram_in("s1T8", (D, SH), f8)
    for li in (0, 1):
        dram_in(f"l{li}_qT8", (D, D), f8)
        dram_in(f"l{li}_kT8", (D, D), f8)
        dram_in(f"l{li}_vT8", (D + 2, VW), f8)
        dram_in(f"l{li}_oT8", (D + 2, D), f8)
        dram_in(f"l{li}_w1T8", (D, D), f8)
        dram_in(f"l{li}_w2T8", (D + 2, D), f8)
    dram_in("spar", (P, 19, ND), f32)
    dram_in("pwT", (S + 1, S), f16)
    dram_in("finT", (2 * D, D), f16)

    outT = nc.dram_tensor("outT", [D, SH], f32, kind="ExternalOutput")
    scr1 = nc.dram_tensor("scr1", [SH * D], f16, kind="Internal")
    scr2 = nc.dram_tensor("scr2", [SH * D], f16, kind="Internal")

    with tile.TileContext(nc) as tc:
        _emit(nc, tc, tile, dt, AF, OP, din, outT, scr1, scr2, make_identity)

    nc.compile()
    return nc


def _emit(nc, tc, tile, dt, AF, OP, din, outT, scr1, scr2, make_identity):
    f32 = dt.float32
    f16 = dt.float16
    f8 = dt.float8e4
    import contextlib

    DR = None
    from concourse import mybir

    DR = mybir.MatmulPerfMode.DoubleRow
    EXS = 0.125 / (WS * WS)  # softmax exp input scale

    es = contextlib.ExitStack()
    with es:
        persist = es.enter_context(tc.tile_pool(name="persist", bufs=1))
        wp = es.enter_context(tc.tile_pool(name="wp", bufs=1))
        wp8 = es.enter_context(tc.tile_pool(name="wp8", bufs=14))
        psA = es.enter_context(tc.tile_pool(name="psA", bufs=2, space="PSUM"))
        psC = es.enter_context(tc.tile_pool(name="psC", bufs=1, space="PSUM"))
        psD = es.enter_context(tc.tile_pool(name="psD", bufs=1, space="PSUM"))
        psS = es.enter_context(tc.tile_pool(name="psS", bufs=2, space="PSUM"))
        expool = es.enter_context(tc.tile_pool(name="expool", bufs=5))
        sqp = es.enter_context(tc.tile_pool(name="sqp", bufs=4))
        rbp = es.enter_context(tc.tile_pool(name="rbp", bufs=4))
        rsp = es.enter_context(tc.tile_pool(name="rsp", bufs=4))

        # --- constants ---
        ident = persist.tile([P, P], f16, name="ident")
        make_identity(nc, ident)
        ones_mean = persist.tile([P, 1], f16, name="ones_mean")
        nc.vector.memset(ones_mean, 1.0 / D)
        ones_var = persist.tile([P, 1], f16, name="ones_var")
        nc.vector.memset(ones_var, 1.0 / (D - 1))
        ones_r128 = persist.tile([1, P], f16, name="ones_r128")
        nc.vector.memset(ones_r128, 1.0)

        # --- small params (biases, LN): one DMA, sliced views ---
        SPAR = persist.tile([P, 19, ND], f32, name="SPAR")
        nc.sync.dma_start(out=SPAR, in_=din["spar"].ap())
        par = {}
        pnames = ("bq", "bk", "bo", "b1", "b2", "ag", "ab", "fg", "fb")
        for li in (0, 1):
            for bi, bn in enumerate(pnames):
                par[f"l{li}_{bn}"] = SPAR[:, 9 * li + bi, :]
        finb = SPAR[:, 18, :]

        # persistent activations
        FEATS = persist.tile([P, ND, S], f16, name="FEATS")
        F8 = persist.tile([P, 8, S], f8, name="F8")
        nc.vector.memset(F8[0:1, 6, :], 1.0)
        nc.vector.memset(F8[0:1, 7, :], 0.0)
        S1T = persist.tile([P, ND, SH], f16, name="S1T")
        nc.sync.dma_start(
            out=S1T, in_=din["s1T"].ap().rearrange("(j p) s -> p j s", p=P)
        )
        S1T8 = persist.tile([P, ND, SH], f8, name="S1T8")
        nc.sync.dma_start(
            out=S1T8, in_=din["s1T8"].ap().rearrange("(j p) s -> p j s", p=P)
        )
        CTX1p = persist.tile([P, ND, SH], f16, name="CTX1p")
        Q1 = persist.tile([P, 2, NP, SH], f8, name="Q1")
        rb_t = persist.tile([P, S], f16, name="rb_t")
        mrb_t = persist.tile([P, S], f16, name="mrb_t")
        # LN small stats rows
        lnt = persist.tile([1, S], f32, name="lnt")
        lnv = persist.tile([1, S], f32, name="lnv")
        lnr16 = persist.tile([1, S], f16, name="lnr16")
        lnmr16 = persist.tile([1, S], f16, name="lnmr16")

        def load_w(dram_h, nk):
            """fp16 weight k-tiles [P, D]."""
            ap = dram_h.ap()
            tiles = []
            for t in range(nk):
                wt = wp.tile([P, D], f16, tag="w", name=f"w_{dram_h.name}_{t}")
                nc.sync.dma_start(out=wt, in_=ap[t * P : (t + 1) * P, :])
                tiles.append(wt)
            return tiles

        def load_w8(dram_h, width):
            """fp8 DoubleRow pair tiles [P, 2, width]."""
            ap = dram_h.ap()
            tiles = []
            for t in range(NP):
                wt = wp8.tile([P, 2, width], f8, tag="w8", name=f"w8_{dram_h.name}_{t}")
                nc.gpsimd.dma_start(
                    out=wt,
                    in_=ap[2 * t * P : (2 * t + 2) * P, :].rearrange(
                        "(two p) w -> p two w", two=2
                    ),
                )
                tiles.append(wt)
            return tiles

        def fm_proj8(x8, w8s, Sx, evac, morder=None, q0=0, wb8=None):
            """fp8 DR projection: out[m] = sum_t w8s[t][:, :, mslice].T (x) x8-pairs.
            wb8: optional [1, 2, D] bias pair contracted with x8 ones planes."""
            cw = min(512, Sx)
            nch = Sx // cw
            for m in morder if morder is not None else range(ND):
                ps = psA.tile([P, Sx], f32, tag="psA", name=f"ps8_m{m}")
                for c in range(nch):
                    sl = slice(q0 + cw * c, q0 + cw * (c + 1))
                    psl = slice(cw * c, cw * (c + 1))
                    for t in range(NP):
                        nc.tensor.matmul(
                            ps[:, psl],
                            w8s[t][:, :, m * P : (m + 1) * P],
                            x8[:, 2 * t : 2 * t + 2, sl],
                            start=(t == 0),
                            stop=(t == NP - 1 and wb8 is None),
                            perf_mode=DR,
                        )
                    if wb8 is not None:
                        nc.tensor.matmul(
                            ps[:, psl],
                            wb8[:, :, m * P : (m + 1) * P],
                            x8[0:1, 6:8, sl],
                            start=False,
                            stop=True,
                            perf_mode=DR,
                        )
                evac(m, ps, q0)

        def v_proj8(x8, vw8, vb8, V8):
            """fp8 DR V projection (row-major out, bias via ones/zero planes)."""
            for st in range(NS):
                ps = psA.tile([P, VW], f32, tag="psA", name=f"vps{st}")
                ssl = slice(st * P, (st + 1) * P)
                for c0, c1 in ((0, 512), (512, VW)):
                    for t in range(NP):
                        nc.tensor.matmul(
                            ps[:, c0:c1],
                            x8[:, 2 * t : 2 * t + 2, ssl],
                            vw8[t][:, :, c0:c1],
                            start=(t == 0),
                            stop=False,
                            perf_mode=DR,
                        )
                    nc.tensor.matmul(
                        ps[:, c0:c1],
                        x8[0:1, 6:8, ssl],
                        vb8[:, :, c0:c1],
                        start=False,
                        stop=True,
                        perf_mode=DR,
                    )
                with nc.allow_low_precision(reason="fp8 V"):
                    nc.vector.tensor_copy(V8[:, st // 2, st % 2, :], ps)

        def attention8(q0, Qt, Kt, V8, CTXdst, interleave=(), SQ=512):
            """Per-head attention over q-columns [q0, q0+SQ), fp8 DoubleRow.
            Head h -> CTXdst partitions 64*(h%2), d-tile h//2. Softmax
            denominators come from the shared ones-block in V8."""
            qsl = slice(q0, q0 + SQ)
            for h in range(H):
                hp, hb = h // 2, 64 * (h % 2)
                pb = 32 * (h % 4)
                j0 = h // 4
                ctx_ps = psC.tile([DH, SQ], f32, tag="psC", name=f"ctx{h}")
                den = None
                for t in range(NS // 2):
                    ex = expool.tile([P, 2, SQ], f8, tag="ex", name=f"ex{h}_{t}")
                    sc = psA.tile([P, 2 * SQ], f32, tag="psA", name=f"sc{h}_{t}")
                    for half in (0, 1):
                        st = 2 * t + half
                        nc.tensor.matmul(
                            sc[:, half * SQ : (half + 1) * SQ],
                            Kt[pb : pb + 32, :, j0, st * P : (st + 1) * P],
                            Qt[pb : pb + 32, :, j0, qsl],
                            start=True,
                            stop=True,
                            perf_mode=DR,
                            tile_position=(pb, 0),
                        )
                    nc.scalar.activation(ex, sc, AF.Exp, scale=EXS)
                    if t == 0:
                        den = psD.tile([DH, SQ], f32, tag="psD", name=f"den{h}")
                    nc.tensor.matmul(
                        ctx_ps,
                        V8[:, t, :, DH * h : DH * (h + 1)],
                        ex,
                        start=(t == 0),
                        stop=(t == NS // 2 - 1),
                        perf_mode=DR,
                    )
                    nc.tensor.matmul(
                        den,
                        V8[:, t, :, D : D + DH],
                        ex,
                        start=(t == 0),
                        stop=(t == NS // 2 - 1),
                        perf_mode=DR,
                    )
                rs = rbp.tile([DH, SQ], f16, tag="rb", name=f"rs{h}")
                with nc.allow_low_precision(reason="softmax recip"):
                    nc.vector.reciprocal(rs, den)
                if h % 2 == 0:
                    with nc.allow_low_precision(reason="ctx norm"):
                        nc.vector.tensor_mul(
                            CTXdst[0:DH, hp, qsl], ctx_ps, rs
                        )
                else:
                    cte = rbp.tile([DH, SQ], f16, tag="rb", name=f"cte{h}")
                    with nc.allow_low_precision(reason="ctx evac"):
                        nc.vector.tensor_copy(cte, ctx_ps)
                        nc.gpsimd.tensor_mul(
                            CTXdst[hb : hb + DH, hp, qsl], cte, rs
                        )
                if h < len(interleave):
                    interleave[h]()

        def layernorm(Zt, q0, Sx, out_fn, tail=False):
            """LN over partitions(d) of Zt[:, :, q0:q0+Sx] (Bessel std).
            tail=True routes square work to Act (idle outside attention)."""
            cw = min(512, Sx)
            nch = Sx // cw
            for c in range(nch):
                sl = slice(q0 + cw * c, q0 + cw * (c + 1))
                sum_ps = psS.tile([1, cw], f32, tag="psS", name=f"lnsum{c}")
                for k in range(ND):
                    nc.tensor.matmul(
                        sum_ps,
                        ones_mean[:, 0:1],
                        Zt[:, k, sl],
                        start=(k == 0),
                        stop=(k == ND - 1),
                    )
                sq_ps = psS.tile([1, cw], f32, tag="psS", name=f"lnsq{c}")
                for k in range(ND):
                    sq = sqp.tile([P, cw], f16, tag="sq", name=f"sq{k}{c}")
                    if tail:
                        nc.scalar.activation(sq, Zt[:, k, sl], AF.Square)
                    else:
                        nc.gpsimd.tensor_mul(sq, Zt[:, k, sl], Zt[:, k, sl])
                    nc.tensor.matmul(
                        sq_ps,
                        ones_var[:, 0:1],
                        sq,
                        start=(k == 0),
                        stop=(k == ND - 1),
                    )
                # var = E[z^2]*D/(D-1) - mean^2*D/(D-1); r = 1/sqrt(var)
                nc.scalar.activation(
                    lnt[:, sl], sum_ps, AF.Square,
                    scale=math.sqrt(D / (D - 1.0)),
                )
                nc.vector.tensor_sub(lnv[:, sl], sq_ps, lnt[:, sl])
                nc.vector.reciprocal(lnv[:, sl], lnv[:, sl])
                nc.scalar.activation(lnr16[:, sl], lnv[:, sl], AF.Sqrt)
                with nc.allow_low_precision(reason="ln mr fp16"):
                    nc.vector.tensor_mul(lnmr16[:, sl], sum_ps, lnr16[:, sl])
            for c in range(nch):
                sl = slice(q0 + cw * c, q0 + cw * (c + 1))
                rb_ps = psS.tile([P, cw], f32, tag="psS", name=f"rbps{c}")
                nc.tensor.matmul(
                    rb_ps, ones_r128, lnr16[0:1, sl], start=True, stop=True
                )
                nc.vector.tensor_copy(rb_t[:, sl], rb_ps)
                mrb_ps = psS.tile([P, cw], f32, tag="psS", name=f"mrbps{c}")
                nc.tensor.matmul(
                    mrb_ps, ones_r128, lnmr16[0:1, sl], start=True, stop=True
                )
                nc.vector.tensor_copy(mrb_t[:, sl], mrb_ps)
                for k in range(ND):
                    t1 = sqp.tile([P, cw], f16, tag="sq", name=f"ap{k}{c}")
                    nc.vector.tensor_mul(t1, Zt[:, k, sl], rb_t[:, sl])
                    nc.vector.tensor_sub(t1, t1, mrb_t[:, sl])
                    out_fn(k, sl, t1)

        # ================= BLOCK 0 (full S, self-attention on src) =========
        with tc.tile_pool(name="b0a", bufs=5) as act6, tc.tile_pool(
            name="b0x", bufs=1
        ) as actx, tc.tile_pool(name="b0q", bufs=2) as qkp, tc.tile_pool(
            name="b0v", bufs=1
        ) as vp0:
            X0 = actx.tile([P, ND, S], f16, tag="x0", name="X0")
            nc.sync.dma_start(
                out=X0,
                in_=din["srcT"].ap().rearrange("(j p) s -> p j s", p=P),
            )
            X8 = actx.tile([P, 8, S], f8, tag="x8", name="X8")
            nc.sync.dma_start(
                out=X8[:, 0:ND, :],
                in_=din["srcT8"].ap().rearrange("(j p) s -> p j s", p=P),
            )
            nc.vector.memset(X8[0:1, 6, :], 1.0)
            nc.vector.memset(X8[0:1, 7, :], 0.0)

            V0 = vp0.tile([P, NS // 2, 2, VW], f8, name="V0")

            # block1 q-projection depends only on inputs: emit first to fill
            # the startup bubble while block0 weights stream in.
            q1w = load_w8(din["l1_qT8"], D)
            bq1 = par["l1_bq"]

            def ev_q1(m, ps, q0):
                with nc.allow_low_precision(reason="fp8 evac"):
                    if m % 2 == 0:
                        nc.vector.tensor_scalar_add(
                            Q1[:, m // NP, m % NP, :], ps, bq1[:, m : m + 1]
                        )
                    else:
                        nc.scalar.activation(
                            Q1[:, m // NP, m % NP, :], ps, AF.Identity,
                            bias=bq1[:, m : m + 1],
                        )

            fm_proj8(S1T8, q1w, SH, ev_q1)

            # --- q/k/v projections (fp8 DR) ---
            Qt0 = qkp.tile([P, 2, NP, S], f8, tag="qk", name="Qt0")
            Kt0 = qkp.tile([P, 2, NP, S], f8, tag="qk", name="Kt0")
            kw = load_w8(din["l0_kT8"], D)
            bk = par["l0_bk"]

            def ev_k(m, ps, q0):
                with nc.allow_low_precision(reason="fp8 evac"):
                    if m % 2 == 0:
                        nc.vector.tensor_scalar_add(
                            Kt0[:, m // NP, m % NP, :], ps, bk[:, m : m + 1]
                        )
                    else:
                        nc.scalar.activation(
                            Kt0[:, m // NP, m % NP, :], ps, AF.Identity,
                            bias=bk[:, m : m + 1],
                        )

            fm_proj8(X8, kw, S, ev_k, morder=[0, 3, 1, 4, 2, 5])

            qw = load_w8(din["l0_qT8"], D)
            bq = par["l0_bq"]

            def ev_q(m, ps, q0):
                with nc.allow_low_precision(reason="fp8 evac"):
                    if m % 2 == 0:
                        nc.vector.tensor_scalar_add(
                            Qt0[:, m // NP, m % NP, :], ps, bq[:, m : m + 1]
                        )
                    else:
                        nc.scalar.activation(
                            Qt0[:, m // NP, m % NP, :], ps, AF.Identity,
                            bias=bq[:, m : m + 1],
                        )

            fm_proj8(X8, qw, S, ev_q, morder=[0, 3, 1, 4, 2, 5])

            vw = load_w8(din["l0_vT8"], VW)
            vb8 = wp8.tile([1, 2, VW], f8, tag="w8", name="vb0")
            nc.sync.dma_start(
                out=vb8,
                in_=din["l0_vT8"].ap()[D : D + 2, :].rearrange(
                    "(two p) w -> p two w", two=2
                ),
            )
            v_proj8(X8, vw, vb8, V0)

            # prefetch all block0 fp16 weights during attention
            ow8 = load_w8(din["l0_oT8"], D)
            ob8 = wp8.tile([1, 2, D], f8, tag="w8", name="ob0")
            nc.sync.dma_start(
                out=ob8,
                in_=din["l0_oT8"].ap()[D : D + 2, :].rearrange(
                    "(two p) w -> p two w", two=2
                ),
            )
            w18 = load_w8(din["l0_w1T8"], D)
            w28 = load_w8(din["l0_w2T8"], D)
            wb28 = wp8.tile([1, 2, D], f8, tag="w8", name="wb20")
            nc.sync.dma_start(
                out=wb28,
                in_=din["l0_w2T8"].ap()[D : D + 2, :].rearrange(
                    "(two p) w -> p two w", two=2
                ),
            )

            CTX0 = act6.tile([P, 8, S], f8, tag="a6", name="CTX0")
            nc.vector.memset(CTX0[0:1, 6, :], 1.0)
            nc.vector.memset(CTX0[0:1, 7, :], 0.0)
            Z0a = act6.tile([P, ND, S], f16, tag="a6", name="Z0a")
            ATT0 = act6.tile([P, ND, S], f16, tag="a6", name="ATT0")
            ATT08 = actx.tile([P, ND, S], f8, tag="x8", name="ATT08")
            H10 = act6.tile([P, 8, S], f8, tag="a6", name="H10")
            nc.vector.memset(H10[0:1, 6, :], 1.0)
            nc.vector.memset(H10[0:1, 7, :], 0.0)
            Z0b = act6.tile([P, ND, S], f16, tag="a6", name="Z0b")

            bo = par["l0_bo"]
            ag, ab = par["l0_ag"], par["l0_ab"]
            b1 = par["l0_b1"]
            b2 = par["l0_b2"]
            fg, fb = par["l0_fg"], par["l0_fb"]

            def fm_proj(x_k, w_tiles, q0, Sx, evac, ms=None):
                cw = min(512, Sx)
                for m in ms if ms is not None else range(ND):
                    ps = psA.tile([P, Sx], f32, tag="psA", name=f"ps_m{m}")
                    for c in range(Sx // cw):
                        sl = slice(q0 + cw * c, q0 + cw * (c + 1))
                        psl = slice(cw * c, cw * (c + 1))
                        for ki, (xk, wk) in enumerate(zip(x_k, w_tiles)):
                            nc.tensor.matmul(
                                ps[:, psl],
                                wk[:, m * P : (m + 1) * P],
                                xk[:, sl],
                                start=(ki == 0),
                                stop=(ki == len(w_tiles) - 1),
                            )
                    evac(m, ps, q0)


            def mk_ev_o(w):
                def ev_o(m, ps, q0):
                    nc.vector.scalar_tensor_tensor(
                        Z0a[:, m, q0 : q0 + w], ps, 1.0 / (WS * WS),
                        X0[:, m, q0 : q0 + w], OP.mult, OP.add,
                    )
                return ev_o

            def out_att(k, sl, t1):
                nc.vector.tensor_scalar(
                    ATT0[:, k, sl], t1, ag[:, k : k + 1], ab[:, k : k + 1],
                    OP.mult, OP.add,
                )
                with nc.allow_low_precision(reason="fp8 att"):
                    nc.gpsimd.tensor_scalar(
                        ATT08[:, k, sl], t1, ag[:, k : k + 1], ab[:, k : k + 1],
                        OP.mult, OP.add,
                    )

            def mk_ev_w1(w):
                def ev_w1(m, ps, q0):
                    with nc.allow_low_precision(reason="fp8 h1"):
                        nc.scalar.activation(
                            H10[:, m, q0 : q0 + w], ps, AF.Gelu,
                            bias=b1[:, m : m + 1], scale=1.0 / WS,
                        )
                return ev_w1

            def mk_ev_w2(w):
                def ev_w2(m, ps, q0):
                    nc.vector.scalar_tensor_tensor(
                        Z0b[:, m, q0 : q0 + w], ps, 1.0 / WS,
                        ATT0[:, m, q0 : q0 + w], OP.mult, OP.add,
                    )
                return ev_w2

            def out_feats(k, sl, t1):
                nc.vector.tensor_scalar(
                    FEATS[:, k, sl], t1, fg[:, k : k + 1], fb[:, k : k + 1],
                    OP.mult, OP.add,
                )
                with nc.allow_low_precision(reason="fp8 feats"):
                    nc.gpsimd.tensor_scalar(
                        F8[:, k, sl], t1, fg[:, k : k + 1], fb[:, k : k + 1],
                        OP.mult, OP.add,
                    )

            CH = 512  # block0 pipeline chunk width

            def chain_pieces(q0, tail=False):
                """12 emit-thunks for the post-attention chain on one chunk."""
                ev_o, ev_w1, ev_w2 = mk_ev_o(CH), mk_ev_w1(CH), mk_ev_w2(CH)
                pieces = []
                for m in range(ND):
                    pieces.append(
                        lambda m=m: fm_proj8(CTX0, ow8, CH, ev_o,
                                             morder=[m], q0=q0, wb8=ob8)
                    )
                pieces.append(lambda: layernorm(Z0a, q0, CH, out_att, tail=tail))
                pieces.append(
                    lambda: fm_proj8(ATT08, w18, CH, ev_w1, morder=[0, 1, 2], q0=q0)
                )
                pieces.append(
                    lambda: fm_proj8(ATT08, w18, CH, ev_w1, morder=[3, 4, 5], q0=q0)
                )
                pieces.append(
                    lambda: fm_proj8(H10, w28, CH, ev_w2, morder=[0, 1, 2],
                                     q0=q0, wb8=wb28)
                )
                pieces.append(
                    lambda: fm_proj8(H10, w28, CH, ev_w2, morder=[3, 4, 5],
                                     q0=q0, wb8=wb28)
                )
                pieces.append(lambda: layernorm(Z0b, q0, CH, out_feats, tail=tail))
                return pieces

            attention8(0, Qt0, Kt0, V0, CTX0, SQ=CH)
            for ci in range(1, S // CH):
                attention8(
                    ci * CH, Qt0, Kt0, V0, CTX0,
                    interleave=chain_pieces((ci - 1) * CH), SQ=CH,
                )
            for piece in chain_pieces(S - CH, tail=True):
                piece()

        # ================= BLOCK 1 (half S on q-side, cross-attention) ======
        with tc.tile_pool(name="b1a", bufs=4) as a6h, tc.tile_pool(
            name="b1b", bufs=1
        ) as a6f, tc.tile_pool(name="b1v", bufs=1) as vp1, tc.tile_pool(
            name="late", bufs=2
        ) as lp, tc.tile_pool(name="wbig", bufs=14) as wb:
            K1 = a6f.tile([P, 2, NP, S], f8, tag="af", name="K1")
            ATT18 = a6f.tile([P, ND, SH], f8, tag="af", name="ATT18")
            CTX1 = a6h.tile([P, 8, SH], f8, tag="ah", name="CTX1")
            nc.vector.memset(CTX1[0:1, 6, :], 1.0)
            nc.vector.memset(CTX1[0:1, 7, :], 0.0)
            Z1a = a6h.tile([P, ND, SH], f16, tag="ah", name="Z1a")
            ATT1 = a6h.tile([P, ND, SH], f16, tag="ah", name="ATT1")
            H11 = a6h.tile([P, 8, SH], f8, tag="ah", name="H11")
            nc.vector.memset(H11[0:1, 6, :], 1.0)
            nc.vector.memset(H11[0:1, 7, :], 0.0)
            Z1b = a6h.tile([P, ND, SH], f16, tag="ah", name="Z1b")
            V1 = vp1.tile([P, NS // 2, 2, VW], f8, name="V1")

            kw1 = load_w8(din["l1_kT8"], D)
            bk1 = par["l1_bk"]

            def ev_k1(m, ps, q0):
                with nc.allow_low_precision(reason="fp8 evac"):
                    if m % 2 == 0:
                        nc.vector.tensor_scalar_add(
                            K1[:, m // NP, m % NP, :], ps, bk1[:, m : m + 1]
                        )
                    else:
                        nc.scalar.activation(
                            K1[:, m // NP, m % NP, :], ps, AF.Identity,
                            bias=bk1[:, m : m + 1],
                        )

            fm_proj8(F8, kw1, S, ev_k1, morder=[0, 3, 1, 4, 2, 5])

            vw1 = load_w8(din["l1_vT8"], VW)
            vb18 = wp8.tile([1, 2, VW], f8, tag="w8", name="vb1")
            nc.sync.dma_start(
                out=vb18,
                in_=din["l1_vT8"].ap()[D : D + 2, :].rearrange(
                    "(two p) w -> p two w", two=2
                ),
            )
            v_proj8(F8, vw1, vb18, V1)

            ow18 = load_w8(din["l1_oT8"], D)
            ob18 = wp8.tile([1, 2, D], f8, tag="w8", name="ob1")
            nc.sync.dma_start(
                out=ob18,
                in_=din["l1_oT8"].ap()[D : D + 2, :].rearrange(
                    "(two p) w -> p two w", two=2
                ),
            )
            w118 = load_w8(din["l1_w1T8"], D)
            w218 = load_w8(din["l1_w2T8"], D)
            wb218 = wp8.tile([1, 2, D], f8, tag="w8", name="wb21")
            nc.sync.dma_start(
                out=wb218,
                in_=din["l1_w2T8"].ap()[D : D + 2, :].rearrange(
                    "(two p) w -> p two w", two=2
                ),
            )

            bo1 = par["l1_bo"]
            ag1, ab1 = par["l1_ag"], par["l1_ab"]
            b11 = par["l1_b1"]
            b21 = par["l1_b2"]
            fg1, fb1 = par["l1_fg"], par["l1_fb"]

            def fm_projh(x_k, w_tiles, evac, ms=None):
                for m in ms if ms is not None else range(ND):
                    ps = psA.tile([P, SH], f32, tag="psA", name=f"psh_m{m}")
                    for ki, (xk, wk) in enumerate(zip(x_k, w_tiles)):
                        nc.tensor.matmul(
                            ps,
                            wk[:, m * P : (m + 1) * P],
                            xk,
                            start=(ki == 0),
                            stop=(ki == len(w_tiles) - 1),
                        )
                    evac(m, ps, 0)


            def ev_o1(m, ps, q0):
                nc.vector.scalar_tensor_tensor(
                    Z1a[:, m, :], ps, 1.0 / (WS * WS), S1T[:, m, :],
                    OP.mult, OP.add,
                )

            def out_att1(k, sl, t1):
                nc.vector.tensor_scalar(
                    ATT1[:, k, sl], t1, ag1[:, k : k + 1], ab1[:, k : k + 1],
                    OP.mult, OP.add,
                )
                with nc.allow_low_precision(reason="fp8 att"):
                    nc.gpsimd.tensor_scalar(
                        ATT18[:, k, sl], t1, ag1[:, k : k + 1], ab1[:, k : k + 1],
                        OP.mult, OP.add,
                    )

            def ev_w11(m, ps, q0):
                with nc.allow_low_precision(reason="fp8 h1"):
                    nc.scalar.activation(
                        H11[:, m, :], ps, AF.Gelu,
                        bias=b11[:, m : m + 1], scale=1.0 / WS,
                    )

            def ev_w21(m, ps, q0):
                nc.vector.scalar_tensor_tensor(
                    Z1b[:, m, :], ps, 1.0 / WS, ATT1[:, m, :],
                    OP.mult, OP.add,
                )

            def out_ctx1(k, sl, t1):
                nc.vector.tensor_scalar(
                    CTX1p[:, k, sl], t1, fg1[:, k : k + 1], fb1[:, k : k + 1],
                    OP.mult, OP.add,
                )

            pieces1 = []
            for m in range(ND):
                pieces1.append(
                    lambda m=m: fm_proj8(CTX1, ow18, SH, ev_o1,
                                         morder=[m], wb8=ob18)
                )
            pieces1.append(lambda: layernorm(Z1a, 0, SH, out_att1, tail=True))
            pieces1.append(
                lambda: fm_proj8(ATT18, w118, SH, ev_w11, morder=[0, 1, 2])
            )
            pieces1.append(
                lambda: fm_proj8(ATT18, w118, SH, ev_w11, morder=[3, 4, 5])
            )
            pieces1.append(
                lambda: fm_proj8(H11, w218, SH, ev_w21, morder=[0, 1, 2], wb8=wb218)
            )
            pieces1.append(
                lambda: fm_proj8(H11, w218, SH, ev_w21, morder=[3, 4, 5], wb8=wb218)
            )
            pieces1.append(lambda: layernorm(Z1b, 0, SH, out_ctx1, tail=True))

            # prefetch pool/final weights during block1 attention
            pw_t = []
            for k in range(NS):
                t = wb.tile([P, S], f16, tag="wb", name=f"pw{k}")
                nc.sync.dma_start(out=t, in_=din["pwT"].ap()[k * P : (k + 1) * P, :])
                pw_t.append(t)
            pwb = wb.tile([1, S], f16, tag="wb", name="pwb")
            nc.sync.dma_start(out=pwb, in_=din["pwT"].ap()[S : S + 1, :])
            fin_t = []
            for k in range(2 * ND):
                t = wb.tile([P, D], f16, tag="wb", name=f"fin{k}")
                nc.sync.dma_start(
                    out=t, in_=din["finT"].ap()[k * P : (k + 1) * P, :]
                )
                fin_t.append(t)

            # 1) transpose CTX1p [768, 512] -> row-major [512, 768], per st
            C1RM = lp.tile([P, SH // P, D], f16, tag="lt", name="C1RM")

            def c1rm_st(st):
                tp = psA.tile([P, D], f16, tag="psA", name=f"t1ps{st}")
                for j in range(ND):
                    nc.tensor.transpose(
                        tp[:, j * P : (j + 1) * P],
                        CTX1p[:, j, st * P : (st + 1) * P],
                        ident,
                    )
                nc.vector.tensor_copy(C1RM[:, st, :], tp)
                nc.sync.dma_start(
                    out=scr1.ap().rearrange("(s d) -> s d", d=D)[
                        st * P : (st + 1) * P, :
                    ],
                    in_=C1RM[:, st, :],
                )

            pieces1.append(
                lambda: [c1rm_st(st) for st in range(SH // P)]
            )

            attention8(0, Q1, K1, V1, CTX1)
            for piece in pieces1:
                piece()

        # ================= POOL + FINAL (merged into block1 scope) ==========
            # 2) read back as M_view rows [384, 1024], transpose to [1024, 384]
            MV = lp.tile([P, 3, S], f16, tag="lt", name="MV")
            v2 = scr1.ap().rearrange("(r c) -> r c", c=S)
            for rt in range(3):
                nc.sync.dma_start(out=MV[:, rt, :], in_=v2[rt * P : (rt + 1) * P, :])
            MVT = lp.tile([P, NS + 1, 3 * P], f16, tag="lt", name="MVT")
            nc.vector.memset(MVT[0:1, NS, :], 1.0)
            for ct in range(NS):
                tp = psS.tile([P, 3 * P], f16, tag="psS", name=f"t2ps{ct}")
                for rt in range(3):
                    nc.tensor.transpose(
                        tp[:, rt * P : (rt + 1) * P],
                        MV[:, rt, ct * P : (ct + 1) * P],
                        ident,
                    )
                nc.vector.tensor_copy(MVT[:, ct, :], tp)
            # 3) pool matmul: out_rm [384, 1024] = M_view @ pw.T + pb
            PRM = lp.tile([P, 3, S], f16, tag="lt", name="PRM")
            for rt in range(3):
                ps = psA.tile([P, S], f32, tag="psA", name=f"plps{rt}")
                for c in range(2):
                    sl = slice(512 * c, 512 * (c + 1))
                    for ki in range(NS + 1):
                        if ki < NS:
                            lhs = MVT[:, ki, rt * P : (rt + 1) * P]
                            rhs = pw_t[ki][:, sl]
                        else:
                            lhs = MVT[0:1, NS, rt * P : (rt + 1) * P]
                            rhs = pwb[:, sl]
                        nc.tensor.matmul(
                            ps[:, sl], lhs, rhs, start=(ki == 0), stop=(ki == NS)
                        )
                nc.vector.tensor_copy(PRM[:, rt, :], ps)
                nc.sync.dma_start(
                    out=scr2.ap().rearrange("(r c) -> r c", c=S)[
                        rt * P : (rt + 1) * P, :
                    ],
                    in_=PRM[:, rt, :],
                )
            # 4) read back as app row-major [512, 768], transpose -> APPT'
            APPRM = lp.tile([P, SH // P, D], f16, tag="lt", name="APPRM")
            v3 = scr2.ap().rearrange("(s d) -> s d", d=D)
            for st in range(SH // P):
                nc.sync.dma_start(
                    out=APPRM[:, st, :], in_=v3[st * P : (st + 1) * P, :]
                )
            APPT = lp.tile([P, ND, SH], f16, tag="lt", name="APPT")
            for j in range(ND):
                tp = psS.tile([P, SH], f16, tag="psS", name=f"t3ps{j}")
                for st in range(SH // P):
                    nc.tensor.transpose(
                        tp[:, st * P : (st + 1) * P],
                        APPRM[:, st, j * P : (j + 1) * P],
                        ident,
                    )
                nc.vector.tensor_copy(APPT[:, j, :], tp)
            # 5) final: out' = finT.T @ [feats_half ; app]
            OUTT = lp.tile([P, ND, SH], f32, tag="lt", name="OUTT")
            for m in range(ND):
                ps = psS.tile([P, SH], f32, tag="psS", name=f"fps{m}")
                for ki in range(2 * ND):
                    rhs = (
                        FEATS[:, ki, 0:SH]
                        if ki < ND
                        else APPT[:, ki - ND, :]
                    )
                    nc.tensor.matmul(
                        ps,
                        fin_t[ki][:, m * P : (m + 1) * P],
                        rhs,
                        start=(ki == 0),
                        stop=(ki == 2 * ND - 1),
                    )
                nc.scalar.activation(
                    OUTT[:, m, :], ps, AF.Identity, bias=finb[:, m : m + 1]
                )
            nc.sync.dma_start(
                out=outT.ap().rearrange("(j p) s -> p j s", p=P), in_=OUTT
            )


def _qk_perm():
    """New feature index for each original (h, dh): head h's 64 dims become
    two 32-row planes at partitions 32*(h%4) and d-tiles h//4, h//4+3."""
    perm = np.empty(D, dtype=np.int64)
    for h in range(H):
        for dh in range(DH):
            j = (h // 4) + NP * (dh // 32)
            p = 32 * (h % 4) + (dh % 32)
            perm[h * DH + dh] = j * P + p
    return perm


def _prep_inputs(inputs):
    import ml_dtypes

    f8 = ml_dtypes.float8_e4m3

    e = np.ascontiguousarray(np.asarray(inputs["e"], dtype=np.float32))
    f = np.ascontiguousarray(np.asarray(inputs["f"], dtype=np.float32))
    wq = np.asarray(inputs["wq"], np.float32)
    wk = np.asarray(inputs["wk"], np.float32)
    wv = np.asarray(inputs["wv"], np.float32)
    wo = np.asarray(inputs["wo"], np.float32)
    bq = np.asarray(inputs["bq"], np.float32)
    bk = np.asarray(inputs["bk"], np.float32)
    bv = np.asarray(inputs["bv"], np.float32)
    bo = np.asarray(inputs["bo"], np.float32)
    ag = np.asarray(inputs["attn_ln_g"], np.float32)
    ab = np.asarray(inputs["attn_ln_b"], np.float32)
    w1 = np.asarray(inputs["ffn_w1"], np.float32)
    b1 = np.asarray(inputs["ffn_b1"], np.float32)
    w2 = np.asarray(inputs["ffn_w2"], np.float32)
    b2 = np.asarray(inputs["ffn_b2"], np.float32)
    fg = np.asarray(inputs["ffn_ln_g"], np.float32)
    fb = np.asarray(inputs["ffn_ln_b"], np.float32)
    pw = np.asarray(inputs["pool_w"], np.float32)
    pb = np.asarray(inputs["pool_b"], np.float32)
    fw = np.asarray(inputs["final_w"], np.float32)
    fnb = np.asarray(inputs["final_b"], np.float32)

    perm = _qk_perm()

    def vec6(v):
        return np.ascontiguousarray(v.reshape(ND, P).T)

    def q8(x):
        return np.ascontiguousarray(x).astype(f8).view(np.uint8)

    in_maps = []
    for c in range(8):
        ti, b, h = c // 4, (c // 2) % 2, c % 2
        src = e if ti == 0 else f
        s1 = f if ti == 0 else e
        own = slice(SH * h, SH * (h + 1))
        oth = slice(SH * (1 - h), SH * (2 - h))
        src_b = src[:, b, :]
        src_perm = np.concatenate([src_b[own], src_b[oth]], axis=0)
        srcT = np.ascontiguousarray(src_perm.T)
        s1Tm = np.ascontiguousarray(s1[own, b, :].T)
        m = {
            "srcT": srcT.astype(np.float16),
            "srcT8": q8(srcT),
            "s1T": s1Tm.astype(np.float16),
            "s1T8": q8(s1Tm),
            "pwT": np.ascontiguousarray(
                np.concatenate([pw[ti].T, pb[ti][None, :]], axis=0)
            ).astype(np.float16),
            "finT": np.ascontiguousarray(fw[ti].T).astype(np.float16),
        }
        spar_list = []
        for li in (0, 1):
            # q/k: transpose, scale by WS, permute output features
            qT = wq[ti, li].T * WS
            kT = wk[ti, li].T * WS
            qTp = np.empty_like(qT)
            qTp[:, perm] = qT
            kTp = np.empty_like(kT)
            kTp[:, perm] = kT
            bqp = np.empty(D, np.float32)
            bqp[perm] = bq[ti, li] * WS
            bkp = np.empty(D, np.float32)
            bkp[perm] = bk[ti, li] * WS
            # v: plain transpose + scale, bias row + zero row; cols [D:D+64]
            # are a ones-block (weights 0, bias 1) producing softmax denoms
            vT8 = np.zeros((D + 2, D + DH), np.float32)
            vT8[0:D, 0:D] = wv[ti, li].T * WS
            vT8[D, 0:D] = bv[ti, li] * WS
            vT8[D, D:] = 1.0
            m.update(
                {
                    f"l{li}_qT8": q8(qTp),
                    f"l{li}_kT8": q8(kTp),
                    f"l{li}_vT8": q8(vT8),
                    f"l{li}_oT8": q8(np.concatenate([
                        wo[ti, li].T * WS,
                        (bo[ti, li] * WS * WS)[None, :],
                        np.zeros((1, D), np.float32),
                    ], axis=0)),
                    f"l{li}_w1T8": q8(w1[ti, li].T * WS),
                    f"l{li}_w2T8": q8(np.concatenate([
                        w2[ti, li].T * WS, (b2[ti, li] * WS)[None, :],
                        np.zeros((1, D), np.float32),
                    ], axis=0)),
                }
            )
            spar_list.extend([
                vec6(bqp), vec6(bkp), vec6(bo[ti, li]), vec6(b1[ti, li]),
                vec6(b2[ti, li]), vec6(ag[ti, li]), vec6(ab[ti, li]),
                vec6(fg[ti, li]), vec6(fb[ti, li]),
            ])
        spar_list.append(vec6(fnb[ti]))
        m["spar"] = np.ascontiguousarray(
            np.stack(spar_list, axis=1), dtype=np.float32
        )
        in_maps.append(m)
    return in_maps


def get_program():
    if "nc" not in _BUILT:
        _BUILT["nc"] = _build_program()
    return _BUILT["nc"]


def kernel(**inputs):
    from concourse.bass_utils import run_bass_kernel_spmd

    nc = get_program()
    in_maps = _prep_inputs(inputs)
    res = run_bass_kernel_spmd(nc, in_maps, core_ids=list(range(8)))
    c_e_f = np.empty((S, B, D), np.float32)
    c_f_e = np.empty((S, B, D), np.float32)
    for c in range(8):
        ti, b, h = c // 4, (c // 2) % 2, c % 2
        dst = c_e_f if ti == 0 else c_f_e
        dst[SH * h : SH * (h + 1), b, :] = res.results[c]["outT"].T
    return c_e_f, c_f_e
